# revision 11
# baseline (speedup 1.0000x reference)
"""CTC batch loss on 8 TRN2 NeuronCores — pure data parallel, log-space DP.

Strategy (v9, ~243us CoreSim vs ~1032us baseline):
- Batch sharded 128 samples/core = SBUF partitions. The 511 sequential DP
  steps split into a forward alpha chain (t=0..255) and a backward beta
  chain (t=511..255, state g = beta+lp) meeting at t*=255. Both chains
  live interleaved in ONE state row: fwd state s at col 2+s, bwd state s
  REVERSED at col 262-s, so every instruction covers both chains at once
  and the bwd shift offsets match the fwd ones (s-1 -> col-1, s-2 -> col-2).
- Every LSE2 is one fused custom DVE op (quadratic-softplus approx,
  e2e rel err 2e-3 vs the 2e-2 gate):
      LSE_QSP(x, y) = max(x,y) + sq(relu(c0 + c1*(max-min)))
  A second fused op folds the bwd label-end injection AND the emission
  add: INJ1(x, lp; c) = max(x, [Idx==c ? 0 : -3e38]) + lp, with c a
  per-partition scalar from a [128,256] table (9999 = off).
- Parity structure: blank states (even s) never take the s-2 skip path.
  One full-row LSE_QSP computes LSE2(state[s], state[s-1]) for BOTH
  parities at once; blanks are then done (one INJ1 on the even stride-2
  sublattice), while labels take a second LSE_QSP against the rep-gated
  skip path (odd stride-2 sublattice) + their own INJ1. The gated skip
  add runs on the otherwise-idle GPSIMD engine in the shadow of the DVE
  ops. Net per step: 4 DVE ops (261+131+130+130 elems) + 1 Pool op; no
  ScalarE, no cross-engine stalls.
- Emission log-probs are host-gathered into per-sublattice streams and
  shipped bf16 (17 MB/core), DMA'd in graduated chunks issued upfront.
- Readout loss = -LSE_s(alpha_255 + beta_255) via a NEG-padded binary
  tree of LSE_QSP ops (no activation tables needed).
- Monkeypatches around two toolchain bugs (drain with >1 sem waits), and
  runs mybir.codegen_inst_isa_subclasses() so custom-DVE InstISA bytes
  are encoded (raw Bass skips that pass -> walrus "ISA wrong length").
"""
import sys

for _p in ("/opt/trn_rl_repo", "/opt/pypackages"):
    if _p not in sys.path:
        sys.path.insert(0, _p)

import numpy as np
import ml_dtypes

import concourse.bass as bass
import concourse.tile as tile
from concourse import mybir
from concourse.bass_utils import run_bass_kernel_spmd

B, T, C, L = 1024, 512, 128, 64
S = 2 * L + 1
NCORES = 8
BL = B // NCORES
EPS = 1e-7
NEG = -30000.0

TW = 264               # state row width: fwd s at col 2+s, bwd s at col 262-s
W = 261                # full-row window: out cols [2, 263)
NLAB = 130             # odd (label) sublattice elements: cols 3,5,...,261
NBLK = 131             # even (blank) sublattice elements: cols 2,4,...,262
LPW = 132              # per-step lp stream stride (both sublattices)
NSTEP = 256
CHUNKS = [4, 12, 16, 32, 32, 32, 32, 32, 32, 32]
assert sum(CHUNKS) == NSTEP
CINJ_OFF = 9999.0

F32 = mybir.dt.float32
BF16 = mybir.dt.bfloat16
ALU = mybir.AluOpType

SP_C0 = 0.8129
SP_C1 = -0.2261
INJ_BIG = -3.0e38

_MAX_WAITS = 1


def _patched_drain_and_barrier(self, tick_clock, wait_clock):
    from concourse.vector_clock import ScopedClock

    drain_inst = self.nc.sync.drain()
    wait_clock.add_sem_waits(
        drain_inst.ins, ScopedClock({None: tick_clock.global_clock})
    )
    si = drain_inst.ins.sync_info
    waits = list(si.on_wait) if si and si.on_wait else []
    if len(waits) > _MAX_WAITS:
        drain_inst.ins.sync_info = mybir.SyncInfo(
            on_wait=waits[:_MAX_WAITS], on_update=list(si.on_update or [])
        )
        for i in range(_MAX_WAITS, len(waits), _MAX_WAITS):
            extra = self.nc.sync.drain()
            extra.ins.sync_info = mybir.SyncInfo(
                on_wait=waits[i:i + _MAX_WAITS], on_update=[]
            )

    self.nc.all_engine_barrier()
    assert self.sems is not None
    popped = self.nc._tile_sem_poison_stack.pop()
    assert popped is self._sem_poison
    self.nc.clear_and_free_semaphores(list(self.sems.allocated().values()))
    self.nc.all_engine_barrier()


tile.TileContext._drain_and_barrier = _patched_drain_and_barrier


def _split_multiwait_bir(ant_bir) -> bytes:
    import json as _json

    bir = _json.loads(ant_bir)
    for f in bir.get("functions", []):
        for blk in f.get("blocks", []):
            out = []
            for ins in blk.get("instructions", []):
                si = ins.get("sync_info")
                waits = (si or {}).get("on_wait") or []
                if len(waits) > 1:
                    for j, w in enumerate(waits[:-1]):
                        out.append({
                            "debug": ins.get("debug", 0),
                            "engine": ins["engine"],
                            "ins": [],
                            "name": f"{ins['name']}_w{j}",
                            "opcode": "Drain",
                            "outs": [],
                            "sync_info": {"on_update": [], "on_wait": [w]},
                        })
                    si["on_wait"] = [waits[-1]]
                out.append(ins)
            blk["instructions"] = out
    return _json.dumps(bir).encode()


def _install_bir_splitter():
    import concourse.bass_utils as _bu
    import concourse.bass2jax as _b2j

    orig = _bu.compile_bir_kernel
    if getattr(orig, "_multiwait_patched", False):
        return

    def patched(ant_bir_str, compile_dir_path, neff_name="file.neff", **kw):
        return orig(_split_multiwait_bir(ant_bir_str), compile_dir_path,
                    neff_name=neff_name, **kw)

    patched._multiwait_patched = True
    _bu.compile_bir_kernel = patched
    if hasattr(_b2j, "compile_bir_kernel"):
        _b2j.compile_bir_kernel = patched


_install_bir_splitter()


def _lse_ref(in0, in1, s0, s1, imm2):
    a = np.asarray(in0, np.float32)
    b = np.asarray(in1, np.float32)
    m = np.maximum(a, b)
    t = m - np.minimum(a, b)
    return (m + np.maximum(s0 + s1 * t, 0.0) ** 2).astype(np.float32)


def _inj1_ref(in0, in1, s0, s1, imm2):
    a = np.asarray(in0, np.float32)
    lp = np.asarray(in1, np.float32)
    k = np.arange(a.shape[-1], dtype=np.float32)[None, :]
    u = k - (s0 if isinstance(s0, float) else np.asarray(s0, np.float32))
    inj = np.minimum(u * u, 1.0) * (s1 if isinstance(s1, float)
                                    else np.asarray(s1, np.float32))
    return (np.maximum(a, inj) + lp).astype(np.float32)


_OPS = None


def _make_ops():
    global _OPS
    if _OPS is not None:
        return _OPS
    from concourse import dve_ops as dops
    from concourse.dve_spec import (Spec, Src0, Src1, C0, C1, One, Idx,
                                    relu, sq, maxx, minn, lower)
    from concourse.dve_spec import _has_src1
    from concourse.dve_uop import DveOpSpec

    def register(name, body, ref):
        for existing in dops.OPS:
            if existing.name == name:
                return existing
        spec = Spec(body=body, reference=ref)
        row = dops._CUSTOM_DVE_ROW_BASE + len(dops.OPS)
        shas = {}
        for ver in ("v3", "v4"):
            uops = lower(spec, ver=ver)
            tmp = DveOpSpec(name=name, opcode=row, uops=uops,
                            rd1_en=_has_src1(spec))
            shas[ver] = tmp.sha(ver)
        op = dops.DveOp(name, spec, subdim=False, uops_sha=shas)
        dops.OPS.append(op)
        dops._SUB_OPCODE_FOR_NAME[name] = row
        dops.CUSTOM_DVE_SPECS[name] = spec
        return op

    m = maxx(Src0, Src1)
    n = minn(Src0, Src1)
    lse_body = m + sq(relu(C0 + C1 * (m - n)))
    lse_op = register("LSE_QSP_ANT", lse_body, _lse_ref)

    # single-column inject + emission add: out = max(Src0, w) + Src1 where
    # w = 0.0 exactly at Idx==C0 and C1=-3e38 elsewhere (C0=9999: no-op).
    u = Idx - C0
    inj1_body = maxx(Src0, minn(sq(u), One) * C1) + Src1
    inj1_op = register("INJ1_ANT", inj1_body, _inj1_ref)

    _OPS = (lse_op, inj1_op)
    return _OPS


_cached_nc = None


def build_bass():
    lse_op, inj1_op = _make_ops()
    nc = bass.Bass()
    lpl_d = nc.declare_dram_parameter("lpl", [BL, NSTEP * LPW], BF16, isOutput=False)
    lpb_d = nc.declare_dram_parameter("lpb", [BL, NSTEP * LPW], BF16, isOutput=False)
    rep_d = nc.declare_dram_parameter("rep", [BL, LPW], F32, isOutput=False)
    x0_d = nc.declare_dram_parameter("x0", [BL, TW], F32, isOutput=False)
    cll_d = nc.declare_dram_parameter("cll", [BL, NSTEP], F32, isOutput=False)
    out_d = nc.declare_dram_parameter("out", [BL, 1], F32, isOutput=True)

    with tile.TileContext(nc) as tc:
        with (
            tc.tile_pool(name="lpp", bufs=1) as lp_pool,
            tc.tile_pool(name="persist", bufs=1) as pp,
        ):
            x_a = pp.tile([BL, TW], F32, tag="x_a")
            x_b = pp.tile([BL, TW], F32, tag="x_b")
            lrx_a = pp.tile([BL, LPW], F32, tag="lrx_a")
            lrx_b = pp.tile([BL, LPW], F32, tag="lrx_b")
            l1t = pp.tile([BL, TW], F32, tag="l1t")
            l2t = pp.tile([BL, LPW], F32, tag="l2t")
            rept = pp.tile([BL, LPW], F32, tag="rept")
            cllt = pp.tile([BL, NSTEP], F32, tag="cllt")
            am = pp.tile([BL, 136], F32, tag="am")
            sc = pp.tile([BL, 176], F32, tag="sc")
            loss = pp.tile([BL, 1], F32, tag="loss")

            nc.vector.memset(x_b[:, :], NEG)
            nc.vector.memset(am[:, :], NEG)
            nc.vector.memset(sc[:, :], NEG)
            nc.sync.dma_start(out=x_a[:, :], in_=x0_d[:, :])
            nc.sync.dma_start(out=rept[:, :], in_=rep_d[:, :])
            nc.sync.dma_start(out=cllt[:, :], in_=cll_d[:, :])
            lpts = []
            lo = 0
            for ci, csz in enumerate(CHUNKS):
                lplt = lp_pool.tile([BL, csz * LPW], BF16, tag=f"lpl{ci}")
                nc.sync.dma_start(out=lplt[:, :],
                                  in_=lpl_d[:, lo * LPW:(lo + csz) * LPW])
                lpbt = lp_pool.tile([BL, csz * LPW], BF16, tag=f"lpb{ci}")
                nc.sync.dma_start(out=lpbt[:, :],
                                  in_=lpb_d[:, lo * LPW:(lo + csz) * LPW])
                lpts.append((lplt, lpbt, lo, csz))
                lo += csz

            xc, xn = x_a, x_b
            lrc, lrn = lrx_a, lrx_b
            for lplt, lpbt, lo, csz in lpts:
                for il in range(csz):
                    i = lo + il
                    # gated label skip path on GPSIMD (odd sublattice): runs
                    # in the shadow of the first two DVE ops below.
                    nc.gpsimd.tensor_add(lrc[:, 0:NLAB], xc[:, 1:261:2],
                                         rept[:, 0:NLAB])
                    # DVE: one full-row LSE2 serves both parities
                    nc.vector._custom_dve(lse_op, out=l1t[:, 2:2 + W],
                                          in0=xc[:, 2:2 + W],
                                          in1=xc[:, 1:1 + W],
                                          s0=SP_C0, s1=SP_C1)
                    # blanks are done: inject + emission on the even cols
                    nc.vector._custom_dve(inj1_op, out=xn[:, 2:263:2],
                                          in0=l1t[:, 2:263:2],
                                          in1=lpbt[:, il * LPW: il * LPW + NBLK],
                                          s0=cllt[:, i:i + 1], s1=INJ_BIG)
                    # labels: second LSE2 against the gated skip path
                    nc.vector._custom_dve(lse_op, out=l2t[:, 0:NLAB],
                                          in0=l1t[:, 3:263:2],
                                          in1=lrc[:, 0:NLAB],
                                          s0=SP_C0, s1=SP_C1)
                    nc.vector._custom_dve(inj1_op, out=xn[:, 3:263:2],
                                          in0=l2t[:, 0:NLAB],
                                          in1=lplt[:, il * LPW: il * LPW + NLAB],
                                          s0=cllt[:, i:i + 1], s1=INJ_BIG)
                    xc, xn = xn, xc
                    lrc, lrn = lrn, lrc

            # readout: alpha[s] at col 2+s, beta[s] at col 262-s
            nc.vector.tensor_add(am[:, 0:S], xc[:, 2:2 + S],
                                 xc[:, 262:133:-1])

            def tree(out_o, in_t, in_o, wlo):
                nc.vector._custom_dve(
                    lse_op, out=sc[:, out_o:out_o + wlo],
                    in0=in_t[:, in_o:in_o + wlo],
                    in1=in_t[:, in_o + wlo:in_o + 2 * wlo],
                    s0=SP_C0, s1=SP_C1)

            tree(0, am, 0, 65)      # 129 -> 65  (am[129]=NEG)
            tree(80, sc, 0, 33)     # 65 -> 33   (sc[65]=NEG)
            tree(120, sc, 80, 17)   # 33 -> 17   (sc[113]=NEG)
            tree(140, sc, 120, 9)   # 17 -> 9    (sc[137]=NEG)
            tree(152, sc, 140, 5)   # 9 -> 5     (sc[149]=NEG)
            tree(160, sc, 152, 3)   # 5 -> 3     (sc[157]=NEG)
            tree(168, sc, 160, 2)   # 3 -> 2     (sc[163]=NEG)
            tree(172, sc, 168, 1)   # 2 -> 1
            nc.vector.tensor_scalar_mul(loss[:, 0:1], sc[:, 172:173], -1.0)
            nc.sync.dma_start(out=out_d[:, :], in_=loss[:, 0:1])
    mybir.codegen_inst_isa_subclasses(nc)
    return nc


def _host_prep(y_pred, labels, input_length, label_length):
    blank = C - 1
    lab = labels.astype(np.int64)
    q_l = np.take_along_axis(y_pred, lab[:, None, :], axis=2)   # [B,T,64]
    lp_l = np.log(q_l.astype(np.float32) + EPS)                 # label lp
    lp_b = np.log(y_pred[:, :, blank].astype(np.float32) + EPS) # [B,T] blank lp
    frozen = np.arange(T)[None, :] >= input_length[:, None]
    lp_l[frozen, :] = 0.0
    lp_b[frozen] = 0.0

    # rep gate for label k vs k-1 (fwd: gates l_{k-1} -> l_k)
    rep = np.full((B, L), 0.0, np.float32)
    rep[:, 1:] = np.where(labels[:, 1:] != labels[:, :-1], 0.0, NEG)
    # rep[:, 0] = 0.0: the gated read hits the NEG pad anyway.

    lens = input_length.astype(np.int64)
    llen = label_length.astype(np.int64)                        # [B] in [32,64]

    # odd-sublattice lp stream: e=0..63 fwd labels (out col 3+2e, s=1+2e),
    # e=64,65 pads, e=66..129 bwd labels (s=259-2e -> k=129-e), lp[510-i]
    lpl = np.full((B, NSTEP, LPW), NEG, np.float32)
    lpl[:, :, 0:64] = lp_l[:, 0:NSTEP, :]
    lpl[:, 0:NSTEP - 1, 66:130] = lp_l[:, 510:255:-1, ::-1]
    lpl[:, NSTEP - 1, 66:130] = 0.0
    lpl = lpl.reshape(B, NSTEP * LPW).astype(ml_dtypes.bfloat16)

    # even-sublattice lp stream: e=0..64 fwd blanks (col 2+2e), e=65 pad
    # (col 132), e=66..130 bwd blanks (col 2+2e = 134..262)
    lpb = np.full((B, NSTEP, LPW), NEG, np.float32)
    lpb[:, :, 0:65] = lp_b[:, 0:NSTEP, None]
    lpb[:, 0:NSTEP - 1, 66:131] = lp_b[:, 510:255:-1, None]
    lpb[:, NSTEP - 1, 66:131] = 0.0
    lpb = lpb.reshape(B, NSTEP * LPW).astype(ml_dtypes.bfloat16)

    # rep stream on the odd sublattice: e=0..63 fwd rep_e; e=64..66 pads;
    # e=67..129 bwd label k=129-e gated by rep_{k+1} = rep_{130-e}
    repc = np.full((B, LPW), NEG, np.float32)
    repc[:, 0:64] = rep
    repc[:, 67:130] = rep[:, 1:64][:, ::-1]

    # initial state: fwd s at col 2+s, bwd s at col 262-s
    x0 = np.full((B, TW), NEG, np.float32)
    x0[:, 2] = 0.0                                              # alpha seed s=0
    bi = np.nonzero(lens == 512)[0]
    x0[bi, 262 - 2 * llen[bi]] = lp_b[bi, 511]                  # s_last (blank)
    x0[bi, 263 - 2 * llen[bi]] = lp_l[bi, 511, llen[bi] - 1]    # s_last-1
    # injection table (shared by both sublattices: element 130-llen)
    cl = np.full((B, NSTEP), CINJ_OFF, np.float32)
    ii = 511 - lens
    has = (ii >= 0) & (ii <= 255)
    bi = np.nonzero(has)[0]
    cl[bi, ii[bi]] = (130 - llen[bi]).astype(np.float32)

    return lpl, lpb, repc, x0, cl


def kernel(y_pred, labels, input_length, label_length):
    global _cached_nc
    y_pred = np.asarray(y_pred, np.float32)
    labels = np.asarray(labels, np.int32)
    input_length = np.asarray(input_length, np.int32)
    label_length = np.asarray(label_length, np.int32)
    lpl, lpb, repc, x0, cl = _host_prep(
        y_pred, labels, input_length, label_length)
    if _cached_nc is None:
        _cached_nc = build_bass()
    in_maps = []
    for i in range(NCORES):
        sl = slice(i * BL, (i + 1) * BL)
        in_maps.append({"lpl": lpl[sl], "lpb": lpb[sl], "rep": repc[sl],
                        "x0": x0[sl], "cll": cl[sl]})
    res = run_bass_kernel_spmd(_cached_nc, in_maps, list(range(NCORES)))
    out = np.concatenate([res.results[i]["out"] for i in range(NCORES)], axis=0)
    return out.astype(np.float32)


# revision 13
# speedup vs baseline: 1.0771x; 1.0771x over previous
"""CTC batch loss on 8 TRN2 NeuronCores — v7: parity-split merged chains.

Like v6 (fwd alpha + bwd beta chains meeting at t*=255, QSP-LSE custom DVE
ops, fused inject+emission op), but the extended-state row is split by
parity: blank states (even s) never take the s-2 skip path, so they need
only an LSE2 + emission (2 instructions over 132 cols) while labels
(odd s) run the full LSE3 path (4 instructions over 131 cols). Total
per-step DVE elements drop from 4x261=1044 to 4x131+2x132=788.

Layout (state row, width 268):
  cols 0,1   pad NEG
  cols 2..65    fwd labels l_k  (k=0..63, s=2k+1)
  cols 66..68   pad
  cols 69..132  bwd labels (reversed): gl_k at col 132-k
  cols 133,134  pad (never written)
  cols 135..199 fwd blanks b_k  (k=0..64, s=2k)
  cols 200,201  pad
  cols 202..266 bwd blanks (reversed): gb_k at col 266-k
  col 267    pad

Recurrences (g = beta + lp for the bwd chain, all QSP-approximated):
  fwd: l_k' = lp_l + LSE3(l_k, b_k, l_{k-1}*rep_k);  b_k' = lp_b + LSE2(b_k, l_{k-1})
  bwd: gl_k' = lp_l + LSE3(gl_k, gb_{k+1}, gl_{k+1}*rep_{k+1});  gb_k' = lp_b + LSE2(gb_k, gl_k)
Both halves of each group share one instruction window; the reversed bwd
layout makes all relative offsets match the fwd ones.
"""
import sys

for _p in ("/opt/trn_rl_repo", "/opt/pypackages"):
    if _p not in sys.path:
        sys.path.insert(0, _p)

import numpy as np
import ml_dtypes

import concourse.bass as bass
import concourse.tile as tile
from concourse import mybir
from concourse.bass_utils import run_bass_kernel_spmd

B, T, C, L = 1024, 512, 128, 64
S = 2 * L + 1
NCORES = 8
BL = B // NCORES
EPS = 1e-7
NEG = -30000.0

TW = 264               # state row width: fwd s at col 2+s, bwd s at col 262-s
W = 261                # full-row window: out cols [2, 263)
NLAB = 130             # odd (label) sublattice elements: cols 3,5,...,261
NBLK = 131             # even (blank) sublattice elements: cols 2,4,...,262
LPW = 132              # per-step lp stream stride (both sublattices)
NSTEP = 256
CHUNKS = [4, 12, 16, 32, 32, 32, 32, 32, 32, 32]
assert sum(CHUNKS) == NSTEP
CINJ_OFF = 9999.0

F32 = mybir.dt.float32
BF16 = mybir.dt.bfloat16
ALU = mybir.AluOpType

SP_C0 = 0.8129
SP_C1 = -0.2261
INJ_BIG = -3.0e38

_MAX_WAITS = 1


def _patched_drain_and_barrier(self, tick_clock, wait_clock):
    from concourse.vector_clock import ScopedClock

    drain_inst = self.nc.sync.drain()
    wait_clock.add_sem_waits(
        drain_inst.ins, ScopedClock({None: tick_clock.global_clock})
    )
    si = drain_inst.ins.sync_info
    waits = list(si.on_wait) if si and si.on_wait else []
    if len(waits) > _MAX_WAITS:
        drain_inst.ins.sync_info = mybir.SyncInfo(
            on_wait=waits[:_MAX_WAITS], on_update=list(si.on_update or [])
        )
        for i in range(_MAX_WAITS, len(waits), _MAX_WAITS):
            extra = self.nc.sync.drain()
            extra.ins.sync_info = mybir.SyncInfo(
                on_wait=waits[i:i + _MAX_WAITS], on_update=[]
            )

    self.nc.all_engine_barrier()
    assert self.sems is not None
    popped = self.nc._tile_sem_poison_stack.pop()
    assert popped is self._sem_poison
    self.nc.clear_and_free_semaphores(list(self.sems.allocated().values()))
    self.nc.all_engine_barrier()


tile.TileContext._drain_and_barrier = _patched_drain_and_barrier


def _split_multiwait_bir(ant_bir) -> bytes:
    import json as _json

    bir = _json.loads(ant_bir)
    for f in bir.get("functions", []):
        for blk in f.get("blocks", []):
            out = []
            for ins in blk.get("instructions", []):
                si = ins.get("sync_info")
                waits = (si or {}).get("on_wait") or []
                if len(waits) > 1:
                    for j, w in enumerate(waits[:-1]):
                        out.append({
                            "debug": ins.get("debug", 0),
                            "engine": ins["engine"],
                            "ins": [],
                            "name": f"{ins['name']}_w{j}",
                            "opcode": "Drain",
                            "outs": [],
                            "sync_info": {"on_update": [], "on_wait": [w]},
                        })
                    si["on_wait"] = [waits[-1]]
                out.append(ins)
            blk["instructions"] = out
    return _json.dumps(bir).encode()


def _install_bir_splitter():
    import concourse.bass_utils as _bu
    import concourse.bass2jax as _b2j

    orig = _bu.compile_bir_kernel
    if getattr(orig, "_multiwait_patched", False):
        return

    def patched(ant_bir_str, compile_dir_path, neff_name="file.neff", **kw):
        return orig(_split_multiwait_bir(ant_bir_str), compile_dir_path,
                    neff_name=neff_name, **kw)

    patched._multiwait_patched = True
    _bu.compile_bir_kernel = patched
    if hasattr(_b2j, "compile_bir_kernel"):
        _b2j.compile_bir_kernel = patched


_install_bir_splitter()


def _lse_ref(in0, in1, s0, s1, imm2):
    a = np.asarray(in0, np.float32)
    b = np.asarray(in1, np.float32)
    m = np.maximum(a, b)
    t = m - np.minimum(a, b)
    return (m + np.maximum(s0 + s1 * t, 0.0) ** 2).astype(np.float32)


def _inj1_ref(in0, in1, s0, s1, imm2):
    a = np.asarray(in0, np.float32)
    lp = np.asarray(in1, np.float32)
    k = np.arange(a.shape[-1], dtype=np.float32)[None, :]
    u = k - (s0 if isinstance(s0, float) else np.asarray(s0, np.float32))
    inj = np.minimum(u * u, 1.0) * (s1 if isinstance(s1, float)
                                    else np.asarray(s1, np.float32))
    return (np.maximum(a, inj) + lp).astype(np.float32)


_OPS = None


def _make_ops():
    global _OPS
    if _OPS is not None:
        return _OPS
    from concourse import dve_ops as dops
    from concourse.dve_spec import (Spec, Src0, Src1, C0, C1, One, Idx,
                                    relu, sq, maxx, minn, lower)
    from concourse.dve_spec import _has_src1
    from concourse.dve_uop import DveOpSpec

    def register(name, body, ref):
        for existing in dops.OPS:
            if existing.name == name:
                return existing
        spec = Spec(body=body, reference=ref)
        row = dops._CUSTOM_DVE_ROW_BASE + len(dops.OPS)
        shas = {}
        for ver in ("v3", "v4"):
            uops = lower(spec, ver=ver)
            tmp = DveOpSpec(name=name, opcode=row, uops=uops,
                            rd1_en=_has_src1(spec))
            shas[ver] = tmp.sha(ver)
        op = dops.DveOp(name, spec, subdim=False, uops_sha=shas)
        dops.OPS.append(op)
        dops._SUB_OPCODE_FOR_NAME[name] = row
        dops.CUSTOM_DVE_SPECS[name] = spec
        return op

    m = maxx(Src0, Src1)
    n = minn(Src0, Src1)
    lse_body = m + sq(relu(C0 + C1 * (m - n)))
    lse_op = register("LSE_QSP_ANT", lse_body, _lse_ref)

    # single-column inject + emission add: out = max(Src0, w) + Src1 where
    # w = 0.0 exactly at Idx==C0 and C1=-3e38 elsewhere (C0=9999: no-op).
    u = Idx - C0
    inj1_body = maxx(Src0, minn(sq(u), One) * C1) + Src1
    inj1_op = register("INJ1_ANT", inj1_body, _inj1_ref)

    _OPS = (lse_op, inj1_op)
    return _OPS


_cached_nc = None


def build_bass():
    lse_op, inj1_op = _make_ops()
    nc = bass.Bass()
    lpl_d = nc.declare_dram_parameter("lpl", [BL, NSTEP * LPW], BF16, isOutput=False)
    lpb_d = nc.declare_dram_parameter("lpb", [BL, NSTEP * LPW], BF16, isOutput=False)
    rep_d = nc.declare_dram_parameter("rep", [BL, LPW], F32, isOutput=False)
    x0_d = nc.declare_dram_parameter("x0", [BL, TW], F32, isOutput=False)
    cll_d = nc.declare_dram_parameter("cll", [BL, NSTEP], F32, isOutput=False)
    cle_d = nc.declare_dram_parameter("cle", [BL, NSTEP], F32, isOutput=False)
    out_d = nc.declare_dram_parameter("out", [BL, 1], F32, isOutput=True)

    with tile.TileContext(nc) as tc:
        with (
            tc.tile_pool(name="lpp", bufs=1) as lp_pool,
            tc.tile_pool(name="persist", bufs=1) as pp,
        ):
            x_a = pp.tile([BL, TW], F32, tag="x_a")
            x_b = pp.tile([BL, TW], F32, tag="x_b")
            lrx_a = pp.tile([BL, LPW], F32, tag="lrx_a")
            lrx_b = pp.tile([BL, LPW], F32, tag="lrx_b")
            l1t = pp.tile([BL, TW], F32, tag="l1t")
            l2t = pp.tile([BL, LPW], F32, tag="l2t")
            rept = pp.tile([BL, LPW], F32, tag="rept")
            cllt = pp.tile([BL, NSTEP], F32, tag="cllt")
            clet = pp.tile([BL, NSTEP], F32, tag="clet")
            am = pp.tile([BL, 136], F32, tag="am")
            sc = pp.tile([BL, 176], F32, tag="sc")
            loss = pp.tile([BL, 1], F32, tag="loss")

            nc.vector.memset(x_b[:, :], NEG)
            nc.vector.memset(am[:, :], NEG)
            nc.vector.memset(sc[:, :], NEG)
            nc.sync.dma_start(out=x_a[:, :], in_=x0_d[:, :])
            nc.sync.dma_start(out=rept[:, :], in_=rep_d[:, :])
            nc.sync.dma_start(out=cllt[:, :], in_=cll_d[:, :])
            nc.sync.dma_start(out=clet[:, :], in_=cle_d[:, :])
            lpts = []
            lo = 0
            for ci, csz in enumerate(CHUNKS):
                # lpb is consumed earlier in each step than lpl
                lpbt = lp_pool.tile([BL, csz * LPW], BF16, tag=f"lpb{ci}")
                nc.sync.dma_start(out=lpbt[:, :],
                                  in_=lpb_d[:, lo * LPW:(lo + csz) * LPW])
                lplt = lp_pool.tile([BL, csz * LPW], BF16, tag=f"lpl{ci}")
                nc.sync.dma_start(out=lplt[:, :],
                                  in_=lpl_d[:, lo * LPW:(lo + csz) * LPW])
                lpts.append((lplt, lpbt, lo, csz))
                lo += csz

            xc, xn = x_a, x_b
            lrc, lrn = lrx_a, lrx_b
            for lplt, lpbt, lo, csz in lpts:
                for il in range(csz):
                    i = lo + il
                    # gated label skip path on GPSIMD (odd sublattice); only
                    # Add/Mult lower to Pool-legal ISA opcodes, so gate by
                    # adding {0, NEG}. Runs in the shadow of the DVE ops.
                    nc.gpsimd.tensor_add(lrc[:, 0:NLAB], xc[:, 1:261:2],
                                         rept[:, 0:NLAB])
                    # DVE: one full-row LSE2 serves both parities
                    nc.vector._custom_dve(lse_op, out=l1t[:, 2:2 + W],
                                          in0=xc[:, 2:2 + W],
                                          in1=xc[:, 1:1 + W],
                                          s0=SP_C0, s1=SP_C1)
                    # fwd-even emission add on GPSIMD (no injections there)
                    nc.gpsimd.tensor_add(xn[:, 2:131:2], l1t[:, 2:131:2],
                                         lpbt[:, il * LPW: il * LPW + 65])
                    # bwd-even: inject + emission on DVE
                    nc.vector._custom_dve(inj1_op, out=xn[:, 132:263:2],
                                          in0=l1t[:, 132:263:2],
                                          in1=lpbt[:, il * LPW + 65:
                                                   il * LPW + NBLK],
                                          s0=clet[:, i:i + 1], s1=INJ_BIG)
                    # labels: second LSE2 against the gated skip path
                    nc.vector._custom_dve(lse_op, out=l2t[:, 0:NLAB],
                                          in0=l1t[:, 3:263:2],
                                          in1=lrc[:, 0:NLAB],
                                          s0=SP_C0, s1=SP_C1)
                    nc.vector._custom_dve(inj1_op, out=xn[:, 3:263:2],
                                          in0=l2t[:, 0:NLAB],
                                          in1=lplt[:, il * LPW: il * LPW + NLAB],
                                          s0=cllt[:, i:i + 1], s1=INJ_BIG)
                    xc, xn = xn, xc
                    lrc, lrn = lrn, lrc

            # readout: alpha[s] at col 2+s, beta[s] at col 262-s
            nc.vector.tensor_add(am[:, 0:S], xc[:, 2:2 + S],
                                 xc[:, 262:133:-1])

            def tree(out_o, in_t, in_o, wlo):
                nc.vector._custom_dve(
                    lse_op, out=sc[:, out_o:out_o + wlo],
                    in0=in_t[:, in_o:in_o + wlo],
                    in1=in_t[:, in_o + wlo:in_o + 2 * wlo],
                    s0=SP_C0, s1=SP_C1)

            tree(0, am, 0, 65)      # 129 -> 65  (am[129]=NEG)
            tree(80, sc, 0, 33)     # 65 -> 33   (sc[65]=NEG)
            tree(120, sc, 80, 17)   # 33 -> 17   (sc[113]=NEG)
            tree(140, sc, 120, 9)   # 17 -> 9    (sc[137]=NEG)
            tree(152, sc, 140, 5)   # 9 -> 5     (sc[149]=NEG)
            tree(160, sc, 152, 3)   # 5 -> 3     (sc[157]=NEG)
            tree(168, sc, 160, 2)   # 3 -> 2     (sc[163]=NEG)
            tree(172, sc, 168, 1)   # 2 -> 1
            nc.vector.tensor_scalar_mul(loss[:, 0:1], sc[:, 172:173], -1.0)
            nc.sync.dma_start(out=out_d[:, :], in_=loss[:, 0:1])
    mybir.codegen_inst_isa_subclasses(nc)
    return nc


def _host_prep(y_pred, labels, input_length, label_length):
    blank = C - 1
    lab = labels.astype(np.int64)
    q_l = np.take_along_axis(y_pred, lab[:, None, :], axis=2)   # [B,T,64]
    lp_l = np.log(q_l.astype(np.float32) + EPS)                 # label lp
    lp_b = np.log(y_pred[:, :, blank].astype(np.float32) + EPS) # [B,T] blank lp
    frozen = np.arange(T)[None, :] >= input_length[:, None]
    lp_l[frozen, :] = 0.0
    lp_b[frozen] = 0.0

    # rep gate for label k vs k-1 (fwd: gates l_{k-1} -> l_k)
    rep = np.full((B, L), 0.0, np.float32)
    rep[:, 1:] = np.where(labels[:, 1:] != labels[:, :-1], 0.0, NEG)
    # rep[:, 0] = 0.0: the gated read hits the NEG pad anyway.

    lens = input_length.astype(np.int64)
    llen = label_length.astype(np.int64)                        # [B] in [32,64]

    # odd-sublattice lp stream: e=0..63 fwd labels (out col 3+2e, s=1+2e),
    # e=64,65 pads, e=66..129 bwd labels (s=259-2e -> k=129-e), lp[510-i]
    lpl = np.full((B, NSTEP, LPW), NEG, np.float32)
    lpl[:, :, 0:64] = lp_l[:, 0:NSTEP, :]
    lpl[:, 0:NSTEP - 1, 66:130] = lp_l[:, 510:255:-1, ::-1]
    lpl[:, NSTEP - 1, 66:130] = 0.0
    lpl = lpl.reshape(B, NSTEP * LPW).astype(ml_dtypes.bfloat16)

    # even-sublattice lp stream: e=0..64 fwd blanks (col 2+2e), e=65 pad
    # (col 132), e=66..130 bwd blanks (col 2+2e = 134..262)
    lpb = np.full((B, NSTEP, LPW), NEG, np.float32)
    lpb[:, :, 0:65] = lp_b[:, 0:NSTEP, None]
    lpb[:, 0:NSTEP - 1, 66:131] = lp_b[:, 510:255:-1, None]
    lpb[:, NSTEP - 1, 66:131] = 0.0
    lpb = lpb.reshape(B, NSTEP * LPW).astype(ml_dtypes.bfloat16)

    # rep stream on the odd sublattice: e=0..63 fwd rep_e; e=64..66 pads;
    # e=67..129 bwd label k=129-e gated by rep_{k+1} = rep_{130-e}
    repc = np.full((B, LPW), NEG, np.float32)
    repc[:, 0:64] = rep
    repc[:, 67:130] = rep[:, 1:64][:, ::-1]

    # initial state: fwd s at col 2+s, bwd s at col 262-s
    x0 = np.full((B, TW), NEG, np.float32)
    x0[:, 2] = 0.0                                              # alpha seed s=0
    bi = np.nonzero(lens == 512)[0]
    x0[bi, 262 - 2 * llen[bi]] = lp_b[bi, 511]                  # s_last (blank)
    x0[bi, 263 - 2 * llen[bi]] = lp_l[bi, 511, llen[bi] - 1]    # s_last-1
    # injection tables: odd sublattice element 130-llen; even sublattice's
    # narrowed bwd window starts at element 65, so its Idx = 65-llen
    cl = np.full((B, NSTEP), CINJ_OFF, np.float32)
    cle = np.full((B, NSTEP), CINJ_OFF, np.float32)
    ii = 511 - lens
    has = (ii >= 0) & (ii <= 255)
    bi = np.nonzero(has)[0]
    cl[bi, ii[bi]] = (130 - llen[bi]).astype(np.float32)
    cle[bi, ii[bi]] = (65 - llen[bi]).astype(np.float32)

    return lpl, lpb, repc, x0, cl, cle


def kernel(y_pred, labels, input_length, label_length):
    global _cached_nc
    y_pred = np.asarray(y_pred, np.float32)
    labels = np.asarray(labels, np.int32)
    input_length = np.asarray(input_length, np.int32)
    label_length = np.asarray(label_length, np.int32)
    lpl, lpb, repc, x0, cl, cle = _host_prep(
        y_pred, labels, input_length, label_length)
    if _cached_nc is None:
        _cached_nc = build_bass()
    in_maps = []
    for i in range(NCORES):
        sl = slice(i * BL, (i + 1) * BL)
        in_maps.append({"lpl": lpl[sl], "lpb": lpb[sl], "rep": repc[sl],
                        "x0": x0[sl], "cll": cl[sl], "cle": cle[sl]})
    res = run_bass_kernel_spmd(_cached_nc, in_maps, list(range(NCORES)))
    out = np.concatenate([res.results[i]["out"] for i in range(NCORES)], axis=0)
    return out.astype(np.float32)


# revision 14
# speedup vs baseline: 1.0795x; 1.0022x over previous
"""CTC batch loss on 8 TRN2 NeuronCores — pure data parallel, log-space DP.

Strategy (v10, ~226us CoreSim vs ~1032us baseline):
- Batch sharded 128 samples/core = SBUF partitions. The 511 sequential DP
  steps split into a forward alpha chain (t=0..255) and a backward beta
  chain (t=511..255, state g = beta+lp) meeting at t*=255. Both chains
  live interleaved in ONE state row (fwd s at col 2+s, bwd s REVERSED at
  col 262-s) so one instruction covers both chains, and the label states
  of both chains share the odd stride-2 sublattice.
- Every LSE2 is one fused custom DVE op (quadratic-softplus approx, e2e
  rel err 2e-3 vs the 2e-2 gate):
      LSE_QSP(x, y) = max(x,y) + sq(relu(c0 + c1*(max-min)))
  A second fused op folds the bwd label-end injection AND the emission
  add: INJ1(x, lp; c) = max(x, [Idx==c ? 0 : -3e38]) + lp, with c a
  per-partition scalar from a [128,256] table (9999 = off).
- Per step: (1) one full-row LSE_QSP computes LSE2(state[s], state[s-1])
  for BOTH parities; (2) blanks (even s, never take the s-2 skip) are
  then done: INJ1 on the bwd-even sublattice, while the fwd-even
  emission add (no injections can land there) runs on the idle GPSIMD
  engine; (3) labels take a second LSE_QSP against the rep-gated skip
  path (computed by GPSIMD in the DVE ops' shadow) + their own INJ1.
  Net: 4 DVE ops (261+66+130+130 elems) + 2 hidden Pool ops per step,
  no ScalarE, no cross-engine stalls. Only Add/Mult lower to Pool-legal
  ISA opcodes, so the skip gate is additive {0, NEG}.
- Emission log-probs host-gathered into per-sublattice bf16 streams
  (17 MB/core), DMA'd in graduated upfront chunks; small tables issue
  from the ACT queue to stay off the SP DMA queue's critical path.
- Readout loss = -LSE_s(alpha_255 + beta_255) via a NEG-padded binary
  tree of LSE_QSP ops (no activation tables needed).
- Monkeypatches around two toolchain bugs (drain with >1 sem waits), and
  runs mybir.codegen_inst_isa_subclasses() so custom-DVE InstISA bytes
  are encoded (raw Bass skips that pass -> walrus "ISA wrong length").
"""
import sys

for _p in ("/opt/trn_rl_repo", "/opt/pypackages"):
    if _p not in sys.path:
        sys.path.insert(0, _p)

import numpy as np
import ml_dtypes

import concourse.bass as bass
import concourse.tile as tile
from concourse import mybir
from concourse.bass_utils import run_bass_kernel_spmd

B, T, C, L = 1024, 512, 128, 64
S = 2 * L + 1
NCORES = 8
BL = B // NCORES
EPS = 1e-7
NEG = -30000.0

TW = 264               # state row width: fwd s at col 2+s, bwd s at col 262-s
W = 261                # full-row window: out cols [2, 263)
NLAB = 130             # odd (label) sublattice elements: cols 3,5,...,261
NBLK = 131             # even (blank) sublattice elements: cols 2,4,...,262
LPW = 132              # per-step lp stream stride (both sublattices)
NSTEP = 256
CHUNKS = [4, 12, 48, 192]
assert sum(CHUNKS) == NSTEP
CINJ_OFF = 9999.0

F32 = mybir.dt.float32
BF16 = mybir.dt.bfloat16
ALU = mybir.AluOpType

SP_C0 = 0.8129
SP_C1 = -0.2261
INJ_BIG = -3.0e38

_MAX_WAITS = 1


def _patched_drain_and_barrier(self, tick_clock, wait_clock):
    from concourse.vector_clock import ScopedClock

    drain_inst = self.nc.sync.drain()
    wait_clock.add_sem_waits(
        drain_inst.ins, ScopedClock({None: tick_clock.global_clock})
    )
    si = drain_inst.ins.sync_info
    waits = list(si.on_wait) if si and si.on_wait else []
    if len(waits) > _MAX_WAITS:
        drain_inst.ins.sync_info = mybir.SyncInfo(
            on_wait=waits[:_MAX_WAITS], on_update=list(si.on_update or [])
        )
        for i in range(_MAX_WAITS, len(waits), _MAX_WAITS):
            extra = self.nc.sync.drain()
            extra.ins.sync_info = mybir.SyncInfo(
                on_wait=waits[i:i + _MAX_WAITS], on_update=[]
            )

    self.nc.all_engine_barrier()
    assert self.sems is not None
    popped = self.nc._tile_sem_poison_stack.pop()
    assert popped is self._sem_poison
    self.nc.clear_and_free_semaphores(list(self.sems.allocated().values()))
    self.nc.all_engine_barrier()


tile.TileContext._drain_and_barrier = _patched_drain_and_barrier


def _split_multiwait_bir(ant_bir) -> bytes:
    import json as _json

    bir = _json.loads(ant_bir)
    for f in bir.get("functions", []):
        for blk in f.get("blocks", []):
            out = []
            for ins in blk.get("instructions", []):
                si = ins.get("sync_info")
                waits = (si or {}).get("on_wait") or []
                if len(waits) > 1:
                    for j, w in enumerate(waits[:-1]):
                        out.append({
                            "debug": ins.get("debug", 0),
                            "engine": ins["engine"],
                            "ins": [],
                            "name": f"{ins['name']}_w{j}",
                            "opcode": "Drain",
                            "outs": [],
                            "sync_info": {"on_update": [], "on_wait": [w]},
                        })
                    si["on_wait"] = [waits[-1]]
                out.append(ins)
            blk["instructions"] = out
    return _json.dumps(bir).encode()


def _install_bir_splitter():
    import concourse.bass_utils as _bu
    import concourse.bass2jax as _b2j

    orig = _bu.compile_bir_kernel
    if getattr(orig, "_multiwait_patched", False):
        return

    def patched(ant_bir_str, compile_dir_path, neff_name="file.neff", **kw):
        return orig(_split_multiwait_bir(ant_bir_str), compile_dir_path,
                    neff_name=neff_name, **kw)

    patched._multiwait_patched = True
    _bu.compile_bir_kernel = patched
    if hasattr(_b2j, "compile_bir_kernel"):
        _b2j.compile_bir_kernel = patched


_install_bir_splitter()


def _lse_ref(in0, in1, s0, s1, imm2):
    a = np.asarray(in0, np.float32)
    b = np.asarray(in1, np.float32)
    m = np.maximum(a, b)
    t = m - np.minimum(a, b)
    return (m + np.maximum(s0 + s1 * t, 0.0) ** 2).astype(np.float32)


def _inj1_ref(in0, in1, s0, s1, imm2):
    a = np.asarray(in0, np.float32)
    lp = np.asarray(in1, np.float32)
    k = np.arange(a.shape[-1], dtype=np.float32)[None, :]
    u = k - (s0 if isinstance(s0, float) else np.asarray(s0, np.float32))
    inj = np.minimum(u * u, 1.0) * (s1 if isinstance(s1, float)
                                    else np.asarray(s1, np.float32))
    return (np.maximum(a, inj) + lp).astype(np.float32)


_OPS = None


def _make_ops():
    global _OPS
    if _OPS is not None:
        return _OPS
    from concourse import dve_ops as dops
    from concourse.dve_spec import (Spec, Src0, Src1, C0, C1, One, Idx,
                                    relu, sq, maxx, minn, lower)
    from concourse.dve_spec import _has_src1
    from concourse.dve_uop import DveOpSpec

    def register(name, body, ref):
        for existing in dops.OPS:
            if existing.name == name:
                return existing
        spec = Spec(body=body, reference=ref)
        row = dops._CUSTOM_DVE_ROW_BASE + len(dops.OPS)
        shas = {}
        for ver in ("v3", "v4"):
            uops = lower(spec, ver=ver)
            tmp = DveOpSpec(name=name, opcode=row, uops=uops,
                            rd1_en=_has_src1(spec))
            shas[ver] = tmp.sha(ver)
        op = dops.DveOp(name, spec, subdim=False, uops_sha=shas)
        dops.OPS.append(op)
        dops._SUB_OPCODE_FOR_NAME[name] = row
        dops.CUSTOM_DVE_SPECS[name] = spec
        return op

    m = maxx(Src0, Src1)
    n = minn(Src0, Src1)
    lse_body = m + sq(relu(C0 + C1 * (m - n)))
    lse_op = register("LSE_QSP_ANT", lse_body, _lse_ref)

    # single-column inject + emission add: out = max(Src0, w) + Src1 where
    # w = 0.0 exactly at Idx==C0 and C1=-3e38 elsewhere (C0=9999: no-op).
    u = Idx - C0
    inj1_body = maxx(Src0, minn(sq(u), One) * C1) + Src1
    inj1_op = register("INJ1_ANT", inj1_body, _inj1_ref)

    _OPS = (lse_op, inj1_op)
    return _OPS


_cached_nc = None


def build_bass():
    lse_op, inj1_op = _make_ops()
    nc = bass.Bass()
    lpl_d = nc.declare_dram_parameter("lpl", [BL, NSTEP * LPW], BF16, isOutput=False)
    lpb_d = nc.declare_dram_parameter("lpb", [BL, NSTEP * LPW], BF16, isOutput=False)
    rep_d = nc.declare_dram_parameter("rep", [BL, LPW], F32, isOutput=False)
    x0_d = nc.declare_dram_parameter("x0", [BL, TW], F32, isOutput=False)
    cll_d = nc.declare_dram_parameter("cll", [BL, NSTEP], F32, isOutput=False)
    cle_d = nc.declare_dram_parameter("cle", [BL, NSTEP], F32, isOutput=False)
    out_d = nc.declare_dram_parameter("out", [BL, 1], F32, isOutput=True)

    with tile.TileContext(nc) as tc:
        with (
            tc.tile_pool(name="lpp", bufs=1) as lp_pool,
            tc.tile_pool(name="persist", bufs=1) as pp,
        ):
            x_a = pp.tile([BL, TW], F32, tag="x_a")
            x_b = pp.tile([BL, TW], F32, tag="x_b")
            lrx_a = pp.tile([BL, LPW], F32, tag="lrx_a")
            lrx_b = pp.tile([BL, LPW], F32, tag="lrx_b")
            l1t = pp.tile([BL, TW], F32, tag="l1t")
            l2t = pp.tile([BL, LPW], F32, tag="l2t")
            rept = pp.tile([BL, LPW], F32, tag="rept")
            cllt = pp.tile([BL, NSTEP], F32, tag="cllt")
            clet = pp.tile([BL, NSTEP], F32, tag="clet")
            am = pp.tile([BL, 136], F32, tag="am")
            sc = pp.tile([BL, 176], F32, tag="sc")
            loss = pp.tile([BL, 1], F32, tag="loss")

            nc.vector.memset(x_b[:, :], NEG)
            nc.vector.memset(am[:, :], NEG)
            nc.vector.memset(sc[:, :], NEG)
            nc.sync.dma_start(out=x_a[:, :], in_=x0_d[:, :])
            # small tables issue from the idle ACT queue so the lp chunk
            # issue on SP isn't serialized behind them
            nc.scalar.dma_start(out=rept[:, :], in_=rep_d[:, :])
            nc.scalar.dma_start(out=cllt[:, :], in_=cll_d[:, :])
            nc.scalar.dma_start(out=clet[:, :], in_=cle_d[:, :])
            lpts = []
            lo = 0
            for ci, csz in enumerate(CHUNKS):
                # lpb is consumed earlier in each step than lpl
                lpbt = lp_pool.tile([BL, csz * LPW], BF16, tag=f"lpb{ci}")
                nc.sync.dma_start(out=lpbt[:, :],
                                  in_=lpb_d[:, lo * LPW:(lo + csz) * LPW])
                lplt = lp_pool.tile([BL, csz * LPW], BF16, tag=f"lpl{ci}")
                nc.sync.dma_start(out=lplt[:, :],
                                  in_=lpl_d[:, lo * LPW:(lo + csz) * LPW])
                lpts.append((lplt, lpbt, lo, csz))
                lo += csz

            xc, xn = x_a, x_b
            lrc, lrn = lrx_a, lrx_b
            for lplt, lpbt, lo, csz in lpts:
                for il in range(csz):
                    i = lo + il
                    # gated label skip path on GPSIMD (odd sublattice); only
                    # Add/Mult lower to Pool-legal ISA opcodes, so gate by
                    # adding {0, NEG}. Runs in the shadow of the DVE ops.
                    nc.gpsimd.tensor_add(lrc[:, 0:NLAB], xc[:, 1:261:2],
                                         rept[:, 0:NLAB])
                    # DVE: one full-row LSE2 serves both parities
                    nc.vector._custom_dve(lse_op, out=l1t[:, 2:2 + W],
                                          in0=xc[:, 2:2 + W],
                                          in1=xc[:, 1:1 + W],
                                          s0=SP_C0, s1=SP_C1)
                    # fwd-even emission add on GPSIMD (no injections there)
                    nc.gpsimd.tensor_add(xn[:, 2:131:2], l1t[:, 2:131:2],
                                         lpbt[:, il * LPW: il * LPW + 65])
                    # bwd-even: inject + emission on DVE
                    nc.vector._custom_dve(inj1_op, out=xn[:, 132:263:2],
                                          in0=l1t[:, 132:263:2],
                                          in1=lpbt[:, il * LPW + 65:
                                                   il * LPW + NBLK],
                                          s0=clet[:, i:i + 1], s1=INJ_BIG)
                    # labels: second LSE2 against the gated skip path
                    nc.vector._custom_dve(lse_op, out=l2t[:, 0:NLAB],
                                          in0=l1t[:, 3:263:2],
                                          in1=lrc[:, 0:NLAB],
                                          s0=SP_C0, s1=SP_C1)
                    nc.vector._custom_dve(inj1_op, out=xn[:, 3:263:2],
                                          in0=l2t[:, 0:NLAB],
                                          in1=lplt[:, il * LPW: il * LPW + NLAB],
                                          s0=cllt[:, i:i + 1], s1=INJ_BIG)
                    xc, xn = xn, xc
                    lrc, lrn = lrn, lrc

            # readout: alpha[s] at col 2+s, beta[s] at col 262-s
            nc.vector.tensor_add(am[:, 0:S], xc[:, 2:2 + S],
                                 xc[:, 262:133:-1])

            def tree(out_o, in_t, in_o, wlo):
                nc.vector._custom_dve(
                    lse_op, out=sc[:, out_o:out_o + wlo],
                    in0=in_t[:, in_o:in_o + wlo],
                    in1=in_t[:, in_o + wlo:in_o + 2 * wlo],
                    s0=SP_C0, s1=SP_C1)

            tree(0, am, 0, 65)      # 129 -> 65  (am[129]=NEG)
            tree(80, sc, 0, 33)     # 65 -> 33   (sc[65]=NEG)
            tree(120, sc, 80, 17)   # 33 -> 17   (sc[113]=NEG)
            tree(140, sc, 120, 9)   # 17 -> 9    (sc[137]=NEG)
            tree(152, sc, 140, 5)   # 9 -> 5     (sc[149]=NEG)
            tree(160, sc, 152, 3)   # 5 -> 3     (sc[157]=NEG)
            tree(168, sc, 160, 2)   # 3 -> 2     (sc[163]=NEG)
            tree(172, sc, 168, 1)   # 2 -> 1
            nc.vector.tensor_scalar_mul(loss[:, 0:1], sc[:, 172:173], -1.0)
            nc.sync.dma_start(out=out_d[:, :], in_=loss[:, 0:1])
    mybir.codegen_inst_isa_subclasses(nc)
    return nc


def _host_prep(y_pred, labels, input_length, label_length):
    blank = C - 1
    lab = labels.astype(np.int64)
    q_l = np.take_along_axis(y_pred, lab[:, None, :], axis=2)   # [B,T,64]
    lp_l = np.log(q_l.astype(np.float32) + EPS)                 # label lp
    lp_b = np.log(y_pred[:, :, blank].astype(np.float32) + EPS) # [B,T] blank lp
    frozen = np.arange(T)[None, :] >= input_length[:, None]
    lp_l[frozen, :] = 0.0
    lp_b[frozen] = 0.0

    # rep gate for label k vs k-1 (fwd: gates l_{k-1} -> l_k)
    rep = np.full((B, L), 0.0, np.float32)
    rep[:, 1:] = np.where(labels[:, 1:] != labels[:, :-1], 0.0, NEG)
    # rep[:, 0] = 0.0: the gated read hits the NEG pad anyway.

    lens = input_length.astype(np.int64)
    llen = label_length.astype(np.int64)                        # [B] in [32,64]

    # odd-sublattice lp stream: e=0..63 fwd labels (out col 3+2e, s=1+2e),
    # e=64,65 pads, e=66..129 bwd labels (s=259-2e -> k=129-e), lp[510-i]
    lpl = np.full((B, NSTEP, LPW), NEG, np.float32)
    lpl[:, :, 0:64] = lp_l[:, 0:NSTEP, :]
    lpl[:, 0:NSTEP - 1, 66:130] = lp_l[:, 510:255:-1, ::-1]
    lpl[:, NSTEP - 1, 66:130] = 0.0
    lpl = lpl.reshape(B, NSTEP * LPW).astype(ml_dtypes.bfloat16)

    # even-sublattice lp stream: e=0..64 fwd blanks (col 2+2e), e=65 pad
    # (col 132), e=66..130 bwd blanks (col 2+2e = 134..262)
    lpb = np.full((B, NSTEP, LPW), NEG, np.float32)
    lpb[:, :, 0:65] = lp_b[:, 0:NSTEP, None]
    lpb[:, 0:NSTEP - 1, 66:131] = lp_b[:, 510:255:-1, None]
    lpb[:, NSTEP - 1, 66:131] = 0.0
    lpb = lpb.reshape(B, NSTEP * LPW).astype(ml_dtypes.bfloat16)

    # rep stream on the odd sublattice: e=0..63 fwd rep_e; e=64..66 pads;
    # e=67..129 bwd label k=129-e gated by rep_{k+1} = rep_{130-e}
    repc = np.full((B, LPW), NEG, np.float32)
    repc[:, 0:64] = rep
    repc[:, 67:130] = rep[:, 1:64][:, ::-1]

    # initial state: fwd s at col 2+s, bwd s at col 262-s
    x0 = np.full((B, TW), NEG, np.float32)
    x0[:, 2] = 0.0                                              # alpha seed s=0
    bi = np.nonzero(lens == 512)[0]
    x0[bi, 262 - 2 * llen[bi]] = lp_b[bi, 511]                  # s_last (blank)
    x0[bi, 263 - 2 * llen[bi]] = lp_l[bi, 511, llen[bi] - 1]    # s_last-1
    # injection tables: odd sublattice element 130-llen; even sublattice's
    # narrowed bwd window starts at element 65, so its Idx = 65-llen
    cl = np.full((B, NSTEP), CINJ_OFF, np.float32)
    cle = np.full((B, NSTEP), CINJ_OFF, np.float32)
    ii = 511 - lens
    has = (ii >= 0) & (ii <= 255)
    bi = np.nonzero(has)[0]
    cl[bi, ii[bi]] = (130 - llen[bi]).astype(np.float32)
    cle[bi, ii[bi]] = (65 - llen[bi]).astype(np.float32)

    return lpl, lpb, repc, x0, cl, cle


def kernel(y_pred, labels, input_length, label_length):
    global _cached_nc
    y_pred = np.asarray(y_pred, np.float32)
    labels = np.asarray(labels, np.int32)
    input_length = np.asarray(input_length, np.int32)
    label_length = np.asarray(label_length, np.int32)
    lpl, lpb, repc, x0, cl, cle = _host_prep(
        y_pred, labels, input_length, label_length)
    if _cached_nc is None:
        _cached_nc = build_bass()
    in_maps = []
    for i in range(NCORES):
        sl = slice(i * BL, (i + 1) * BL)
        in_maps.append({"lpl": lpl[sl], "lpb": lpb[sl], "rep": repc[sl],
                        "x0": x0[sl], "cll": cl[sl], "cle": cle[sl]})
    res = run_bass_kernel_spmd(_cached_nc, in_maps, list(range(NCORES)))
    out = np.concatenate([res.results[i]["out"] for i in range(NCORES)], axis=0)
    return out.astype(np.float32)


# revision 17
# speedup vs baseline: 1.0899x; 1.0096x over previous
"""CTC batch loss on 8 TRN2 NeuronCores — pure data parallel, log-space DP.

Strategy (v11, ~223us CoreSim vs ~1032us baseline):
- Batch sharded 128 samples/core = SBUF partitions. The 511 sequential DP
  steps split into a forward alpha chain (t=0..255) and a backward beta
  chain (t=511..255, state g = beta+lp) meeting at t*=255. Both chains
  live interleaved in ONE state row (fwd s at col 2+s, bwd s REVERSED at
  col 262-s) so one instruction covers both chains, and the label states
  of both chains share the odd stride-2 sublattice.
- Every LSE2 is one fused custom DVE op (quadratic-softplus approx, e2e
  rel err 2e-3 vs the 2e-2 gate):
      LSE_QSP(x, y) = max(x,y) + sq(relu(c0 + c1*(max-min)))
  A second fused op folds the bwd label-end injection AND the emission
  add: INJ1(x, lp; c) = max(x, [Idx==c ? 0 : -3e38]) + lp, with c a
  per-partition scalar from a [128,256] table (9999 = off).
- Per step: (1) one full-row LSE_QSP computes LSE2(state[s], state[s-1])
  for BOTH parities; (2) blanks (even s, never take the s-2 skip) are
  then done: INJ1 on the bwd-even sublattice, while the fwd-even
  emission add (no injections can land there) runs on the idle GPSIMD
  engine; (3) labels take a second LSE_QSP against the rep-gated skip
  path (computed by GPSIMD in the DVE ops' shadow) + their own INJ1.
  Net: 4 DVE ops (261+66+130+130 elems) + 2 hidden Pool ops per step,
  no ScalarE, no cross-engine stalls. Only Add/Mult lower to Pool-legal
  ISA opcodes, so the skip gate is additive {0, NEG}. For the first ~28
  steps every window is narrowed to the live wavefront (fwd mass at
  s <= 2i+1, bwd mass at s >= 63-2i), growing 2 cols/step to full width.
- Emission log-probs host-gathered into per-sublattice bf16 streams
  (17 MB/core), DMA'd in graduated upfront chunks; small tables issue
  from the ACT queue to stay off the SP DMA queue's critical path.
- Readout loss = -LSE_s(alpha_255 + beta_255) via a NEG-padded binary
  tree of LSE_QSP ops (no activation tables needed).
- Monkeypatches around two toolchain bugs (drain with >1 sem waits), and
  runs mybir.codegen_inst_isa_subclasses() so custom-DVE InstISA bytes
  are encoded (raw Bass skips that pass -> walrus "ISA wrong length").
"""
import sys

for _p in ("/opt/trn_rl_repo", "/opt/pypackages"):
    if _p not in sys.path:
        sys.path.insert(0, _p)

import numpy as np
import ml_dtypes

import concourse.bass as bass
import concourse.tile as tile
from concourse import mybir
from concourse.bass_utils import run_bass_kernel_spmd

B, T, C, L = 1024, 512, 128, 64
S = 2 * L + 1
NCORES = 8
BL = B // NCORES
EPS = 1e-7
NEG = -30000.0

TW = 264               # state row width: fwd s at col 2+s, bwd s at col 262-s
W = 261                # full-row window: out cols [2, 263)
NLAB = 130             # odd (label) sublattice elements: cols 3,5,...,261
NBLK = 131             # even (blank) sublattice elements: cols 2,4,...,262
LPW = 132              # per-step lp stream stride (both sublattices)
NSTEP = 256
CHUNKS = [4, 12, 48, 192]
assert sum(CHUNKS) == NSTEP
CINJ_OFF = 9999.0

F32 = mybir.dt.float32
BF16 = mybir.dt.bfloat16
ALU = mybir.AluOpType

SP_C0 = 0.8129
SP_C1 = -0.2261
INJ_BIG = -3.0e38

_MAX_WAITS = 1


def _patched_drain_and_barrier(self, tick_clock, wait_clock):
    from concourse.vector_clock import ScopedClock

    drain_inst = self.nc.sync.drain()
    wait_clock.add_sem_waits(
        drain_inst.ins, ScopedClock({None: tick_clock.global_clock})
    )
    si = drain_inst.ins.sync_info
    waits = list(si.on_wait) if si and si.on_wait else []
    if len(waits) > _MAX_WAITS:
        drain_inst.ins.sync_info = mybir.SyncInfo(
            on_wait=waits[:_MAX_WAITS], on_update=list(si.on_update or [])
        )
        for i in range(_MAX_WAITS, len(waits), _MAX_WAITS):
            extra = self.nc.sync.drain()
            extra.ins.sync_info = mybir.SyncInfo(
                on_wait=waits[i:i + _MAX_WAITS], on_update=[]
            )

    self.nc.all_engine_barrier()
    assert self.sems is not None
    popped = self.nc._tile_sem_poison_stack.pop()
    assert popped is self._sem_poison
    self.nc.clear_and_free_semaphores(list(self.sems.allocated().values()))
    self.nc.all_engine_barrier()


tile.TileContext._drain_and_barrier = _patched_drain_and_barrier


def _split_multiwait_bir(ant_bir) -> bytes:
    import json as _json

    bir = _json.loads(ant_bir)
    for f in bir.get("functions", []):
        for blk in f.get("blocks", []):
            out = []
            for ins in blk.get("instructions", []):
                si = ins.get("sync_info")
                waits = (si or {}).get("on_wait") or []
                if len(waits) > 1:
                    for j, w in enumerate(waits[:-1]):
                        out.append({
                            "debug": ins.get("debug", 0),
                            "engine": ins["engine"],
                            "ins": [],
                            "name": f"{ins['name']}_w{j}",
                            "opcode": "Drain",
                            "outs": [],
                            "sync_info": {"on_update": [], "on_wait": [w]},
                        })
                    si["on_wait"] = [waits[-1]]
                out.append(ins)
            blk["instructions"] = out
    return _json.dumps(bir).encode()


def _install_bir_splitter():
    import concourse.bass_utils as _bu
    import concourse.bass2jax as _b2j

    orig = _bu.compile_bir_kernel
    if getattr(orig, "_multiwait_patched", False):
        return

    def patched(ant_bir_str, compile_dir_path, neff_name="file.neff", **kw):
        return orig(_split_multiwait_bir(ant_bir_str), compile_dir_path,
                    neff_name=neff_name, **kw)

    patched._multiwait_patched = True
    _bu.compile_bir_kernel = patched
    if hasattr(_b2j, "compile_bir_kernel"):
        _b2j.compile_bir_kernel = patched


_install_bir_splitter()


def _lse_ref(in0, in1, s0, s1, imm2):
    a = np.asarray(in0, np.float32)
    b = np.asarray(in1, np.float32)
    m = np.maximum(a, b)
    t = m - np.minimum(a, b)
    return (m + np.maximum(s0 + s1 * t, 0.0) ** 2).astype(np.float32)


def _inj1_ref(in0, in1, s0, s1, imm2):
    a = np.asarray(in0, np.float32)
    lp = np.asarray(in1, np.float32)
    k = np.arange(a.shape[-1], dtype=np.float32)[None, :]
    u = k - (s0 if isinstance(s0, float) else np.asarray(s0, np.float32))
    inj = np.minimum(u * u, 1.0) * (s1 if isinstance(s1, float)
                                    else np.asarray(s1, np.float32))
    return (np.maximum(a, inj) + lp).astype(np.float32)


_OPS = None


def _make_ops():
    global _OPS
    if _OPS is not None:
        return _OPS
    from concourse import dve_ops as dops
    from concourse.dve_spec import (Spec, Src0, Src1, C0, C1, One, Idx,
                                    relu, sq, maxx, minn, lower)
    from concourse.dve_spec import _has_src1
    from concourse.dve_uop import DveOpSpec

    def register(name, body, ref):
        for existing in dops.OPS:
            if existing.name == name:
                return existing
        spec = Spec(body=body, reference=ref)
        row = dops._CUSTOM_DVE_ROW_BASE + len(dops.OPS)
        shas = {}
        for ver in ("v3", "v4"):
            uops = lower(spec, ver=ver)
            tmp = DveOpSpec(name=name, opcode=row, uops=uops,
                            rd1_en=_has_src1(spec))
            shas[ver] = tmp.sha(ver)
        op = dops.DveOp(name, spec, subdim=False, uops_sha=shas)
        dops.OPS.append(op)
        dops._SUB_OPCODE_FOR_NAME[name] = row
        dops.CUSTOM_DVE_SPECS[name] = spec
        return op

    m = maxx(Src0, Src1)
    n = minn(Src0, Src1)
    lse_body = m + sq(relu(C0 + C1 * (m - n)))
    lse_op = register("LSE_QSP_ANT", lse_body, _lse_ref)

    # single-column inject + emission add: out = max(Src0, w) + Src1 where
    # w = 0.0 exactly at Idx==C0 and C1=-3e38 elsewhere (C0=9999: no-op).
    u = Idx - C0
    inj1_body = maxx(Src0, minn(sq(u), One) * C1) + Src1
    inj1_op = register("INJ1_ANT", inj1_body, _inj1_ref)

    _OPS = (lse_op, inj1_op)
    return _OPS


_cached_nc = None


def build_bass():
    lse_op, inj1_op = _make_ops()
    nc = bass.Bass()
    lpl_d = nc.declare_dram_parameter("lpl", [BL, NSTEP * LPW], BF16, isOutput=False)
    lpb_d = nc.declare_dram_parameter("lpb", [BL, NSTEP * LPW], BF16, isOutput=False)
    rep_d = nc.declare_dram_parameter("rep", [BL, LPW], F32, isOutput=False)
    x0_d = nc.declare_dram_parameter("x0", [BL, TW], F32, isOutput=False)
    cll_d = nc.declare_dram_parameter("cll", [BL, NSTEP], F32, isOutput=False)
    cle_d = nc.declare_dram_parameter("cle", [BL, NSTEP], F32, isOutput=False)
    out_d = nc.declare_dram_parameter("out", [BL, 1], F32, isOutput=True)

    with tile.TileContext(nc) as tc:
        with (
            tc.tile_pool(name="lpp", bufs=1) as lp_pool,
            tc.tile_pool(name="persist", bufs=1) as pp,
        ):
            x_a = pp.tile([BL, TW], F32, tag="x_a")
            x_b = pp.tile([BL, TW], F32, tag="x_b")
            lrx_a = pp.tile([BL, LPW], F32, tag="lrx_a")
            lrx_b = pp.tile([BL, LPW], F32, tag="lrx_b")
            l1t = pp.tile([BL, TW], F32, tag="l1t")
            l2t = pp.tile([BL, LPW], F32, tag="l2t")
            rept = pp.tile([BL, LPW], F32, tag="rept")
            cllt = pp.tile([BL, NSTEP], F32, tag="cllt")
            clet = pp.tile([BL, NSTEP], F32, tag="clet")
            am = pp.tile([BL, 136], F32, tag="am")
            sc = pp.tile([BL, 176], F32, tag="sc")
            loss = pp.tile([BL, 1], F32, tag="loss")

            nc.vector.memset(x_b[:, :], NEG)
            nc.vector.memset(am[:, :], NEG)
            nc.vector.memset(sc[:, :], NEG)
            nc.sync.dma_start(out=x_a[:, :], in_=x0_d[:, :])
            # small tables issue from the idle ACT queue so the lp chunk
            # issue on SP isn't serialized behind them
            nc.scalar.dma_start(out=rept[:, :], in_=rep_d[:, :])
            nc.scalar.dma_start(out=cllt[:, :], in_=cll_d[:, :])
            nc.scalar.dma_start(out=clet[:, :], in_=cle_d[:, :])
            lpts = []
            lo = 0
            for ci, csz in enumerate(CHUNKS):
                # lpb is consumed earlier in each step than lpl
                lpbt = lp_pool.tile([BL, csz * LPW], BF16, tag=f"lpb{ci}")
                nc.sync.dma_start(out=lpbt[:, :],
                                  in_=lpb_d[:, lo * LPW:(lo + csz) * LPW])
                lplt = lp_pool.tile([BL, csz * LPW], BF16, tag=f"lpl{ci}")
                nc.sync.dma_start(out=lplt[:, :],
                                  in_=lpl_d[:, lo * LPW:(lo + csz) * LPW])
                lpts.append((lplt, lpbt, lo, csz))
                lo += csz

            xc, xn = x_a, x_b
            lrc, lrn = lrx_a, lrx_b
            for lplt, lpbt, lo, csz in lpts:
                for il in range(csz):
                    i = lo + il
                    # early-step wavefront narrowing: fwd mass lives at
                    # s <= 2i+1 (cols <= 3+2i) and bwd mass at s >= 63-2i
                    # (cols <= 199+2i), so all windows can stop at We and
                    # grow by 2 cols/step until full (unwritten xn cols keep
                    # stale NEG-ish values, which read as -inf).
                    We = min(2 + W, 206 + 2 * i)
                    ne = len(range(132, We, 2))
                    no = len(range(3, We, 2))
                    # gated label skip path on GPSIMD (odd sublattice); only
                    # Add/Mult lower to Pool-legal ISA opcodes, so gate by
                    # adding {0, NEG}. Runs in the shadow of the DVE ops.
                    nc.gpsimd.tensor_add(lrc[:, 0:no], xc[:, 1:We - 2:2],
                                         rept[:, 0:no])
                    # DVE: one LSE2 serves both parities
                    nc.vector._custom_dve(lse_op, out=l1t[:, 2:We],
                                          in0=xc[:, 2:We],
                                          in1=xc[:, 1:We - 1],
                                          s0=SP_C0, s1=SP_C1)
                    # fwd-even emission add on GPSIMD (no injections there)
                    nc.gpsimd.tensor_add(xn[:, 2:131:2], l1t[:, 2:131:2],
                                         lpbt[:, il * LPW: il * LPW + 65])
                    # bwd-even: inject + emission on DVE
                    nc.vector._custom_dve(inj1_op, out=xn[:, 132:We:2],
                                          in0=l1t[:, 132:We:2],
                                          in1=lpbt[:, il * LPW + 65:
                                                   il * LPW + 65 + ne],
                                          s0=clet[:, i:i + 1], s1=INJ_BIG)
                    # labels: second LSE2 against the gated skip path
                    nc.vector._custom_dve(lse_op, out=l2t[:, 0:no],
                                          in0=l1t[:, 3:We:2],
                                          in1=lrc[:, 0:no],
                                          s0=SP_C0, s1=SP_C1)
                    nc.vector._custom_dve(inj1_op, out=xn[:, 3:We:2],
                                          in0=l2t[:, 0:no],
                                          in1=lplt[:, il * LPW: il * LPW + no],
                                          s0=cllt[:, i:i + 1], s1=INJ_BIG)
                    xc, xn = xn, xc
                    lrc, lrn = lrn, lrc

            # readout: alpha[s] at col 2+s, beta[s] at col 262-s
            nc.vector.tensor_add(am[:, 0:S], xc[:, 2:2 + S],
                                 xc[:, 262:133:-1])

            def tree(out_o, in_t, in_o, wlo):
                nc.vector._custom_dve(
                    lse_op, out=sc[:, out_o:out_o + wlo],
                    in0=in_t[:, in_o:in_o + wlo],
                    in1=in_t[:, in_o + wlo:in_o + 2 * wlo],
                    s0=SP_C0, s1=SP_C1)

            tree(0, am, 0, 65)      # 129 -> 65  (am[129]=NEG)
            tree(80, sc, 0, 33)     # 65 -> 33   (sc[65]=NEG)
            tree(120, sc, 80, 17)   # 33 -> 17   (sc[113]=NEG)
            tree(140, sc, 120, 9)   # 17 -> 9    (sc[137]=NEG)
            tree(152, sc, 140, 5)   # 9 -> 5     (sc[149]=NEG)
            tree(160, sc, 152, 3)   # 5 -> 3     (sc[157]=NEG)
            tree(168, sc, 160, 2)   # 3 -> 2     (sc[163]=NEG)
            tree(172, sc, 168, 1)   # 2 -> 1
            nc.vector.tensor_scalar_mul(loss[:, 0:1], sc[:, 172:173], -1.0)
            nc.sync.dma_start(out=out_d[:, :], in_=loss[:, 0:1])
    mybir.codegen_inst_isa_subclasses(nc)
    return nc


def _host_prep(y_pred, labels, input_length, label_length):
    blank = C - 1
    lab = labels.astype(np.int64)
    q_l = np.take_along_axis(y_pred, lab[:, None, :], axis=2)   # [B,T,64]
    lp_l = np.log(q_l.astype(np.float32) + EPS)                 # label lp
    lp_b = np.log(y_pred[:, :, blank].astype(np.float32) + EPS) # [B,T] blank lp
    frozen = np.arange(T)[None, :] >= input_length[:, None]
    lp_l[frozen, :] = 0.0
    lp_b[frozen] = 0.0

    # rep gate for label k vs k-1 (fwd: gates l_{k-1} -> l_k)
    rep = np.full((B, L), 0.0, np.float32)
    rep[:, 1:] = np.where(labels[:, 1:] != labels[:, :-1], 0.0, NEG)
    # rep[:, 0] = 0.0: the gated read hits the NEG pad anyway.

    lens = input_length.astype(np.int64)
    llen = label_length.astype(np.int64)                        # [B] in [32,64]

    # odd-sublattice lp stream: e=0..63 fwd labels (out col 3+2e, s=1+2e),
    # e=64,65 pads, e=66..129 bwd labels (s=259-2e -> k=129-e), lp[510-i]
    lpl = np.full((B, NSTEP, LPW), NEG, np.float32)
    lpl[:, :, 0:64] = lp_l[:, 0:NSTEP, :]
    lpl[:, 0:NSTEP - 1, 66:130] = lp_l[:, 510:255:-1, ::-1]
    lpl[:, NSTEP - 1, 66:130] = 0.0
    lpl = lpl.reshape(B, NSTEP * LPW).astype(ml_dtypes.bfloat16)

    # even-sublattice lp stream: e=0..64 fwd blanks (col 2+2e), e=65 pad
    # (col 132), e=66..130 bwd blanks (col 2+2e = 134..262)
    lpb = np.full((B, NSTEP, LPW), NEG, np.float32)
    lpb[:, :, 0:65] = lp_b[:, 0:NSTEP, None]
    lpb[:, 0:NSTEP - 1, 66:131] = lp_b[:, 510:255:-1, None]
    lpb[:, NSTEP - 1, 66:131] = 0.0
    lpb = lpb.reshape(B, NSTEP * LPW).astype(ml_dtypes.bfloat16)

    # rep stream on the odd sublattice: e=0..63 fwd rep_e; e=64..66 pads;
    # e=67..129 bwd label k=129-e gated by rep_{k+1} = rep_{130-e}
    repc = np.full((B, LPW), NEG, np.float32)
    repc[:, 0:64] = rep
    repc[:, 67:130] = rep[:, 1:64][:, ::-1]

    # initial state: fwd s at col 2+s, bwd s at col 262-s
    x0 = np.full((B, TW), NEG, np.float32)
    x0[:, 2] = 0.0                                              # alpha seed s=0
    bi = np.nonzero(lens == 512)[0]
    x0[bi, 262 - 2 * llen[bi]] = lp_b[bi, 511]                  # s_last (blank)
    x0[bi, 263 - 2 * llen[bi]] = lp_l[bi, 511, llen[bi] - 1]    # s_last-1
    # injection tables: odd sublattice element 130-llen; even sublattice's
    # narrowed bwd window starts at element 65, so its Idx = 65-llen
    cl = np.full((B, NSTEP), CINJ_OFF, np.float32)
    cle = np.full((B, NSTEP), CINJ_OFF, np.float32)
    ii = 511 - lens
    has = (ii >= 0) & (ii <= 255)
    bi = np.nonzero(has)[0]
    cl[bi, ii[bi]] = (130 - llen[bi]).astype(np.float32)
    cle[bi, ii[bi]] = (65 - llen[bi]).astype(np.float32)

    return lpl, lpb, repc, x0, cl, cle


def kernel(y_pred, labels, input_length, label_length):
    global _cached_nc
    y_pred = np.asarray(y_pred, np.float32)
    labels = np.asarray(labels, np.int32)
    input_length = np.asarray(input_length, np.int32)
    label_length = np.asarray(label_length, np.int32)
    lpl, lpb, repc, x0, cl, cle = _host_prep(
        y_pred, labels, input_length, label_length)
    if _cached_nc is None:
        _cached_nc = build_bass()
    in_maps = []
    for i in range(NCORES):
        sl = slice(i * BL, (i + 1) * BL)
        in_maps.append({"lpl": lpl[sl], "lpb": lpb[sl], "rep": repc[sl],
                        "x0": x0[sl], "cll": cl[sl], "cle": cle[sl]})
    res = run_bass_kernel_spmd(_cached_nc, in_maps, list(range(NCORES)))
    out = np.concatenate([res.results[i]["out"] for i in range(NCORES)], axis=0)
    return out.astype(np.float32)


# revision 20
# speedup vs baseline: 1.0954x; 1.0051x over previous
"""CTC batch loss on 8 TRN2 NeuronCores — pure data parallel, log-space DP.

Strategy (v15, ~222us CoreSim vs ~1032us baseline):
- Batch sharded 128 samples/core = SBUF partitions. The 511 sequential DP
  steps split into a forward alpha chain (t=0..255) and a backward beta
  chain (t=511..255, state g = beta+lp) meeting at t*=255. Both chains
  live interleaved in ONE state row (fwd s at col 2+s, bwd s REVERSED at
  col 262-s) so one instruction covers both chains, and the label states
  of both chains share the odd stride-2 sublattice.
- Every LSE2 is one fused custom DVE op (quadratic-softplus approx, e2e
  rel err 2e-3 vs the 2e-2 gate):
      LSE_QSP(x, y) = max(x,y) + sq(relu(c0 + c1*(max-min)))
  A second fused op folds the bwd label-end injection AND the emission
  add: INJ1(x, lp; c) = max(x, [Idx==c ? 0 : -3e38]) + lp, with c a
  per-partition scalar from a [128,256] table (9999 = off).
- Per step: (1) one full-row LSE_QSP computes LSE2(state[s], state[s-1])
  for BOTH parities; (2) blanks (even s, never take the s-2 skip) are
  then done: INJ1 on the bwd-even sublattice, while the fwd-even
  emission add (no injections can land there) runs on the idle GPSIMD
  engine; (3) labels take a second LSE_QSP against the rep-gated skip
  path (computed by GPSIMD in the DVE ops' shadow) + their own INJ1.
  Net: 4 DVE ops (261+66+130+130 elems) + 2 hidden Pool ops per step,
  no ScalarE, no cross-engine stalls. Only Add/Mult lower to Pool-legal
  ISA opcodes, so the skip gate is additive {0, NEG}. Early steps are
  wavefront-narrowed (fwd mass at s<=2i+1, bwd at s>=63-2i): windows
  stop at We=min(263,206+2i), and while the dead middle between the fwd
  wavefront and the bwd region is wider than one instruction-init buys,
  the full-row LSE2 runs as two split-range ops instead of one.
- Emission log-probs host-gathered into per-sublattice bf16 streams
  (17 MB/core), DMA'd in graduated upfront chunks; small tables issue
  from the ACT queue to stay off the SP DMA queue's critical path.
- Readout loss = -LSE_s(alpha_255 + beta_255) via a NEG-padded binary
  tree of LSE_QSP ops (no activation tables needed).
- Monkeypatches around two toolchain bugs (drain with >1 sem waits), and
  runs mybir.codegen_inst_isa_subclasses() so custom-DVE InstISA bytes
  are encoded (raw Bass skips that pass -> walrus "ISA wrong length").
"""
import sys

for _p in ("/opt/trn_rl_repo", "/opt/pypackages"):
    if _p not in sys.path:
        sys.path.insert(0, _p)

import numpy as np
import ml_dtypes

import concourse.bass as bass
import concourse.tile as tile
from concourse import mybir
from concourse.bass_utils import run_bass_kernel_spmd

B, T, C, L = 1024, 512, 128, 64
S = 2 * L + 1
NCORES = 8
BL = B // NCORES
EPS = 1e-7
NEG = -30000.0

TW = 264               # state row width: fwd s at col 2+s, bwd s at col 262-s
W = 261                # full-row window: out cols [2, 263)
NLAB = 130             # odd (label) sublattice elements: cols 3,5,...,261
NBLK = 131             # even (blank) sublattice elements: cols 2,4,...,262
LPW = 132              # per-step lp stream stride (both sublattices)
NSTEP = 256
CHUNKS = [4, 12, 48, 192]
assert sum(CHUNKS) == NSTEP
CINJ_OFF = 9999.0

F32 = mybir.dt.float32
BF16 = mybir.dt.bfloat16
ALU = mybir.AluOpType

SP_C0 = 0.8129
SP_C1 = -0.2261
INJ_BIG = -3.0e38

_MAX_WAITS = 1


def _patched_drain_and_barrier(self, tick_clock, wait_clock):
    from concourse.vector_clock import ScopedClock

    drain_inst = self.nc.sync.drain()
    wait_clock.add_sem_waits(
        drain_inst.ins, ScopedClock({None: tick_clock.global_clock})
    )
    si = drain_inst.ins.sync_info
    waits = list(si.on_wait) if si and si.on_wait else []
    if len(waits) > _MAX_WAITS:
        drain_inst.ins.sync_info = mybir.SyncInfo(
            on_wait=waits[:_MAX_WAITS], on_update=list(si.on_update or [])
        )
        for i in range(_MAX_WAITS, len(waits), _MAX_WAITS):
            extra = self.nc.sync.drain()
            extra.ins.sync_info = mybir.SyncInfo(
                on_wait=waits[i:i + _MAX_WAITS], on_update=[]
            )

    self.nc.all_engine_barrier()
    assert self.sems is not None
    popped = self.nc._tile_sem_poison_stack.pop()
    assert popped is self._sem_poison
    self.nc.clear_and_free_semaphores(list(self.sems.allocated().values()))
    self.nc.all_engine_barrier()


tile.TileContext._drain_and_barrier = _patched_drain_and_barrier


def _split_multiwait_bir(ant_bir) -> bytes:
    import json as _json

    bir = _json.loads(ant_bir)
    for f in bir.get("functions", []):
        for blk in f.get("blocks", []):
            out = []
            for ins in blk.get("instructions", []):
                si = ins.get("sync_info")
                waits = (si or {}).get("on_wait") or []
                if len(waits) > 1:
                    for j, w in enumerate(waits[:-1]):
                        out.append({
                            "debug": ins.get("debug", 0),
                            "engine": ins["engine"],
                            "ins": [],
                            "name": f"{ins['name']}_w{j}",
                            "opcode": "Drain",
                            "outs": [],
                            "sync_info": {"on_update": [], "on_wait": [w]},
                        })
                    si["on_wait"] = [waits[-1]]
                out.append(ins)
            blk["instructions"] = out
    return _json.dumps(bir).encode()


def _install_bir_splitter():
    import concourse.bass_utils as _bu
    import concourse.bass2jax as _b2j

    orig = _bu.compile_bir_kernel
    if getattr(orig, "_multiwait_patched", False):
        return

    def patched(ant_bir_str, compile_dir_path, neff_name="file.neff", **kw):
        return orig(_split_multiwait_bir(ant_bir_str), compile_dir_path,
                    neff_name=neff_name, **kw)

    patched._multiwait_patched = True
    _bu.compile_bir_kernel = patched
    if hasattr(_b2j, "compile_bir_kernel"):
        _b2j.compile_bir_kernel = patched


_install_bir_splitter()


def _lse_ref(in0, in1, s0, s1, imm2):
    a = np.asarray(in0, np.float32)
    b = np.asarray(in1, np.float32)
    m = np.maximum(a, b)
    t = m - np.minimum(a, b)
    return (m + np.maximum(s0 + s1 * t, 0.0) ** 2).astype(np.float32)


def _inj1_ref(in0, in1, s0, s1, imm2):
    a = np.asarray(in0, np.float32)
    lp = np.asarray(in1, np.float32)
    k = np.arange(a.shape[-1], dtype=np.float32)[None, :]
    u = k - (s0 if isinstance(s0, float) else np.asarray(s0, np.float32))
    inj = np.minimum(u * u, 1.0) * (s1 if isinstance(s1, float)
                                    else np.asarray(s1, np.float32))
    return (np.maximum(a, inj) + lp).astype(np.float32)


_OPS = None


def _make_ops():
    global _OPS
    if _OPS is not None:
        return _OPS
    from concourse import dve_ops as dops
    from concourse.dve_spec import (Spec, Src0, Src1, C0, C1, One, Idx,
                                    relu, sq, maxx, minn, lower)
    from concourse.dve_spec import _has_src1
    from concourse.dve_uop import DveOpSpec

    def register(name, body, ref):
        for existing in dops.OPS:
            if existing.name == name:
                return existing
        spec = Spec(body=body, reference=ref)
        row = dops._CUSTOM_DVE_ROW_BASE + len(dops.OPS)
        shas = {}
        for ver in ("v3", "v4"):
            uops = lower(spec, ver=ver)
            tmp = DveOpSpec(name=name, opcode=row, uops=uops,
                            rd1_en=_has_src1(spec))
            shas[ver] = tmp.sha(ver)
        op = dops.DveOp(name, spec, subdim=False, uops_sha=shas)
        dops.OPS.append(op)
        dops._SUB_OPCODE_FOR_NAME[name] = row
        dops.CUSTOM_DVE_SPECS[name] = spec
        return op

    m = maxx(Src0, Src1)
    n = minn(Src0, Src1)
    lse_body = m + sq(relu(C0 + C1 * (m - n)))
    lse_op = register("LSE_QSP_ANT", lse_body, _lse_ref)

    # single-column inject + emission add: out = max(Src0, w) + Src1 where
    # w = 0.0 exactly at Idx==C0 and C1=-3e38 elsewhere (C0=9999: no-op).
    u = Idx - C0
    inj1_body = maxx(Src0, minn(sq(u), One) * C1) + Src1
    inj1_op = register("INJ1_ANT", inj1_body, _inj1_ref)

    _OPS = (lse_op, inj1_op)
    return _OPS


_cached_nc = None


def build_bass():
    lse_op, inj1_op = _make_ops()
    nc = bass.Bass()
    lpl_d = nc.declare_dram_parameter("lpl", [BL, NSTEP * LPW], BF16, isOutput=False)
    lpb_d = nc.declare_dram_parameter("lpb", [BL, NSTEP * LPW], BF16, isOutput=False)
    rep_d = nc.declare_dram_parameter("rep", [BL, LPW], F32, isOutput=False)
    x0_d = nc.declare_dram_parameter("x0", [BL, TW], F32, isOutput=False)
    cll_d = nc.declare_dram_parameter("cll", [BL, NSTEP], F32, isOutput=False)
    cle_d = nc.declare_dram_parameter("cle", [BL, NSTEP], F32, isOutput=False)
    out_d = nc.declare_dram_parameter("out", [BL, 1], F32, isOutput=True)

    with tile.TileContext(nc) as tc:
        with (
            tc.tile_pool(name="lpp", bufs=1) as lp_pool,
            tc.tile_pool(name="persist", bufs=1) as pp,
        ):
            x_a = pp.tile([BL, TW], F32, tag="x_a")
            x_b = pp.tile([BL, TW], F32, tag="x_b")
            lrx_a = pp.tile([BL, LPW], F32, tag="lrx_a")
            lrx_b = pp.tile([BL, LPW], F32, tag="lrx_b")
            l1t = pp.tile([BL, TW], F32, tag="l1t")
            l2t = pp.tile([BL, LPW], F32, tag="l2t")
            rept = pp.tile([BL, LPW], F32, tag="rept")
            cllt = pp.tile([BL, NSTEP], F32, tag="cllt")
            clet = pp.tile([BL, NSTEP], F32, tag="clet")
            am = pp.tile([BL, 136], F32, tag="am")
            sc = pp.tile([BL, 176], F32, tag="sc")
            loss = pp.tile([BL, 1], F32, tag="loss")

            nc.vector.memset(x_b[:, :], NEG)
            nc.vector.memset(l1t[:, :], NEG)
            nc.vector.memset(am[:, :], NEG)
            nc.vector.memset(sc[:, :], NEG)
            nc.sync.dma_start(out=x_a[:, :], in_=x0_d[:, :])
            # small tables issue from the idle ACT queue so the lp chunk
            # issue on SP isn't serialized behind them
            nc.scalar.dma_start(out=rept[:, :], in_=rep_d[:, :])
            nc.scalar.dma_start(out=cllt[:, :], in_=cll_d[:, :])
            nc.scalar.dma_start(out=clet[:, :], in_=cle_d[:, :])
            lpts = []
            lo = 0
            for ci, csz in enumerate(CHUNKS):
                # lpb is consumed earlier in each step than lpl
                lpbt = lp_pool.tile([BL, csz * LPW], BF16, tag=f"lpb{ci}")
                nc.sync.dma_start(out=lpbt[:, :],
                                  in_=lpb_d[:, lo * LPW:(lo + csz) * LPW])
                lplt = lp_pool.tile([BL, csz * LPW], BF16, tag=f"lpl{ci}")
                nc.sync.dma_start(out=lplt[:, :],
                                  in_=lpl_d[:, lo * LPW:(lo + csz) * LPW])
                lpts.append((lplt, lpbt, lo, csz))
                lo += csz

            xc, xn = x_a, x_b
            lrc, lrn = lrx_a, lrx_b
            for lplt, lpbt, lo, csz in lpts:
                for il in range(csz):
                    i = lo + il
                    # early-step wavefront narrowing: fwd mass lives at
                    # s <= 2i+1 (cols <= 3+2i) and bwd mass at s >= 63-2i
                    # (cols <= 199+2i), so all windows can stop at We and
                    # grow by 2 cols/step until full (unwritten xn cols keep
                    # stale NEG-ish values, which read as -inf).
                    We = min(2 + W, 206 + 2 * i)
                    ne = len(range(132, We, 2))
                    no = len(range(3, We, 2))
                    # gated label skip path on GPSIMD (odd sublattice); only
                    # Add/Mult lower to Pool-legal ISA opcodes, so gate by
                    # adding {0, NEG}. Runs in the shadow of the DVE ops.
                    nc.gpsimd.tensor_add(lrc[:, 0:no], xc[:, 1:We - 2:2],
                                         rept[:, 0:no])
                    # DVE: one LSE2 serves both parities. While the fwd
                    # wavefront (s <= 2i+1, cols <= 3+2i) is far from the
                    # bwd region, split around the dead middle: the saved
                    # elements beat the extra instruction init for i<=33.
                    Wf = 7 + 2 * i
                    if 132 - Wf > 58:
                        nc.vector._custom_dve(lse_op, out=l1t[:, 2:Wf],
                                              in0=xc[:, 2:Wf],
                                              in1=xc[:, 1:Wf - 1],
                                              s0=SP_C0, s1=SP_C1)
                        nc.vector._custom_dve(lse_op, out=l1t[:, 132:We],
                                              in0=xc[:, 132:We],
                                              in1=xc[:, 131:We - 1],
                                              s0=SP_C0, s1=SP_C1)
                    else:
                        nc.vector._custom_dve(lse_op, out=l1t[:, 2:We],
                                              in0=xc[:, 2:We],
                                              in1=xc[:, 1:We - 1],
                                              s0=SP_C0, s1=SP_C1)
                    # fwd-even emission add on GPSIMD (no injections there)
                    nc.gpsimd.tensor_add(xn[:, 2:131:2], l1t[:, 2:131:2],
                                         lpbt[:, il * LPW: il * LPW + 65])
                    # bwd-even: inject + emission on DVE
                    nc.vector._custom_dve(inj1_op, out=xn[:, 132:We:2],
                                          in0=l1t[:, 132:We:2],
                                          in1=lpbt[:, il * LPW + 65:
                                                   il * LPW + 65 + ne],
                                          s0=clet[:, i:i + 1], s1=INJ_BIG)
                    # labels: second LSE2 against the gated skip path
                    nc.vector._custom_dve(lse_op, out=l2t[:, 0:no],
                                          in0=l1t[:, 3:We:2],
                                          in1=lrc[:, 0:no],
                                          s0=SP_C0, s1=SP_C1)
                    nc.vector._custom_dve(inj1_op, out=xn[:, 3:We:2],
                                          in0=l2t[:, 0:no],
                                          in1=lplt[:, il * LPW: il * LPW + no],
                                          s0=cllt[:, i:i + 1], s1=INJ_BIG)
                    xc, xn = xn, xc
                    lrc, lrn = lrn, lrc

            # readout: alpha[s] at col 2+s, beta[s] at col 262-s
            nc.vector.tensor_add(am[:, 0:S], xc[:, 2:2 + S],
                                 xc[:, 262:133:-1])

            def tree(out_o, in_t, in_o, wlo):
                nc.vector._custom_dve(
                    lse_op, out=sc[:, out_o:out_o + wlo],
                    in0=in_t[:, in_o:in_o + wlo],
                    in1=in_t[:, in_o + wlo:in_o + 2 * wlo],
                    s0=SP_C0, s1=SP_C1)

            tree(0, am, 0, 65)      # 129 -> 65  (am[129]=NEG)
            tree(80, sc, 0, 33)     # 65 -> 33   (sc[65]=NEG)
            tree(120, sc, 80, 17)   # 33 -> 17   (sc[113]=NEG)
            tree(140, sc, 120, 9)   # 17 -> 9    (sc[137]=NEG)
            tree(152, sc, 140, 5)   # 9 -> 5     (sc[149]=NEG)
            tree(160, sc, 152, 3)   # 5 -> 3     (sc[157]=NEG)
            tree(168, sc, 160, 2)   # 3 -> 2     (sc[163]=NEG)
            tree(172, sc, 168, 1)   # 2 -> 1
            nc.vector.tensor_scalar_mul(loss[:, 0:1], sc[:, 172:173], -1.0)
            nc.sync.dma_start(out=out_d[:, :], in_=loss[:, 0:1])
    mybir.codegen_inst_isa_subclasses(nc)
    return nc


def _host_prep(y_pred, labels, input_length, label_length):
    blank = C - 1
    lab = labels.astype(np.int64)
    q_l = np.take_along_axis(y_pred, lab[:, None, :], axis=2)   # [B,T,64]
    lp_l = np.log(q_l.astype(np.float32) + EPS)                 # label lp
    lp_b = np.log(y_pred[:, :, blank].astype(np.float32) + EPS) # [B,T] blank lp
    frozen = np.arange(T)[None, :] >= input_length[:, None]
    lp_l[frozen, :] = 0.0
    lp_b[frozen] = 0.0

    # rep gate for label k vs k-1 (fwd: gates l_{k-1} -> l_k)
    rep = np.full((B, L), 0.0, np.float32)
    rep[:, 1:] = np.where(labels[:, 1:] != labels[:, :-1], 0.0, NEG)
    # rep[:, 0] = 0.0: the gated read hits the NEG pad anyway.

    lens = input_length.astype(np.int64)
    llen = label_length.astype(np.int64)                        # [B] in [32,64]

    # odd-sublattice lp stream: e=0..63 fwd labels (out col 3+2e, s=1+2e),
    # e=64,65 pads, e=66..129 bwd labels (s=259-2e -> k=129-e), lp[510-i]
    lpl = np.full((B, NSTEP, LPW), NEG, np.float32)
    lpl[:, :, 0:64] = lp_l[:, 0:NSTEP, :]
    lpl[:, 0:NSTEP - 1, 66:130] = lp_l[:, 510:255:-1, ::-1]
    lpl[:, NSTEP - 1, 66:130] = 0.0
    lpl = lpl.reshape(B, NSTEP * LPW).astype(ml_dtypes.bfloat16)

    # even-sublattice lp stream: e=0..64 fwd blanks (col 2+2e), e=65 pad
    # (col 132), e=66..130 bwd blanks (col 2+2e = 134..262)
    lpb = np.full((B, NSTEP, LPW), NEG, np.float32)
    lpb[:, :, 0:65] = lp_b[:, 0:NSTEP, None]
    lpb[:, 0:NSTEP - 1, 66:131] = lp_b[:, 510:255:-1, None]
    lpb[:, NSTEP - 1, 66:131] = 0.0
    lpb = lpb.reshape(B, NSTEP * LPW).astype(ml_dtypes.bfloat16)

    # rep stream on the odd sublattice: e=0..63 fwd rep_e; e=64..66 pads;
    # e=67..129 bwd label k=129-e gated by rep_{k+1} = rep_{130-e}
    repc = np.full((B, LPW), NEG, np.float32)
    repc[:, 0:64] = rep
    repc[:, 67:130] = rep[:, 1:64][:, ::-1]

    # initial state: fwd s at col 2+s, bwd s at col 262-s
    x0 = np.full((B, TW), NEG, np.float32)
    x0[:, 2] = 0.0                                              # alpha seed s=0
    bi = np.nonzero(lens == 512)[0]
    x0[bi, 262 - 2 * llen[bi]] = lp_b[bi, 511]                  # s_last (blank)
    x0[bi, 263 - 2 * llen[bi]] = lp_l[bi, 511, llen[bi] - 1]    # s_last-1
    # injection tables: odd sublattice element 130-llen; even sublattice's
    # narrowed bwd window starts at element 65, so its Idx = 65-llen
    cl = np.full((B, NSTEP), CINJ_OFF, np.float32)
    cle = np.full((B, NSTEP), CINJ_OFF, np.float32)
    ii = 511 - lens
    has = (ii >= 0) & (ii <= 255)
    bi = np.nonzero(has)[0]
    cl[bi, ii[bi]] = (130 - llen[bi]).astype(np.float32)
    cle[bi, ii[bi]] = (65 - llen[bi]).astype(np.float32)

    return lpl, lpb, repc, x0, cl, cle


def kernel(y_pred, labels, input_length, label_length):
    global _cached_nc
    y_pred = np.asarray(y_pred, np.float32)
    labels = np.asarray(labels, np.int32)
    input_length = np.asarray(input_length, np.int32)
    label_length = np.asarray(label_length, np.int32)
    lpl, lpb, repc, x0, cl, cle = _host_prep(
        y_pred, labels, input_length, label_length)
    if _cached_nc is None:
        _cached_nc = build_bass()
    in_maps = []
    for i in range(NCORES):
        sl = slice(i * BL, (i + 1) * BL)
        in_maps.append({"lpl": lpl[sl], "lpb": lpb[sl], "rep": repc[sl],
                        "x0": x0[sl], "cll": cl[sl], "cle": cle[sl]})
    res = run_bass_kernel_spmd(_cached_nc, in_maps, list(range(NCORES)))
    out = np.concatenate([res.results[i]["out"] for i in range(NCORES)], axis=0)
    return out.astype(np.float32)


# revision 22
# speedup vs baseline: 1.0967x; 1.0012x over previous
"""CTC batch loss on 8 TRN2 NeuronCores — pure data parallel, log-space DP.

Strategy (v16, ~221.7us CoreSim vs ~1032us baseline):
- Batch sharded 128 samples/core = SBUF partitions. The 511 sequential DP
  steps split into a forward alpha chain (t=0..255) and a backward beta
  chain (t=511..255, state g = beta+lp) meeting at t*=255. Both chains
  live interleaved in ONE state row (fwd s at col 2+s, bwd s REVERSED at
  col 262-s) so one instruction covers both chains, and the label states
  of both chains share the odd stride-2 sublattice.
- Every LSE2 is one fused custom DVE op (quadratic-softplus approx, e2e
  rel err 2e-3 vs the 2e-2 gate):
      LSE_QSP(x, y) = max(x,y) + sq(relu(c0 + c1*(max-min)))
  A second fused op folds the bwd label-end injection AND the emission
  add: INJ1(x, lp; c) = max(x, [Idx==c ? 0 : -3e38]) + lp, with c a
  per-partition scalar from a [128,256] table (9999 = off).
- Per step: (1) one full-row LSE_QSP computes LSE2(state[s], state[s-1])
  for BOTH parities; (2) blanks (even s, never take the s-2 skip) are
  then done: INJ1 on the bwd-even sublattice, while the fwd-even
  emission add (no injections can land there) runs on the idle GPSIMD
  engine; (3) labels take a second LSE_QSP against the rep-gated skip
  path (computed by GPSIMD in the DVE ops' shadow) + their own INJ1.
  Net: 4 DVE ops (261+66+130+130 elems) + 2 hidden Pool ops per step,
  no ScalarE, no cross-engine stalls. Only Add/Mult lower to Pool-legal
  ISA opcodes, so the skip gate is additive {0, NEG}. Early steps are
  wavefront-narrowed (fwd mass at s<=2i+1, bwd at s>=63-2i): windows
  stop at We=min(263,206+2i), and while the dead middle between the fwd
  wavefront and the bwd region is wider than one instruction-init buys,
  the full-row LSE2 runs as two split-range ops instead of one.
- Emission log-probs host-gathered into per-sublattice bf16 streams
  (17 MB/core), DMA'd in graduated upfront chunks; small tables issue
  from the ACT queue to stay off the SP DMA queue's critical path.
- Readout loss = -LSE_s(alpha_255 + beta_255) via a NEG-padded binary
  tree of LSE_QSP ops (no activation tables needed).
- Monkeypatches around two toolchain bugs (drain with >1 sem waits), and
  runs mybir.codegen_inst_isa_subclasses() so custom-DVE InstISA bytes
  are encoded (raw Bass skips that pass -> walrus "ISA wrong length").
"""
import sys

for _p in ("/opt/trn_rl_repo", "/opt/pypackages"):
    if _p not in sys.path:
        sys.path.insert(0, _p)

import numpy as np
import ml_dtypes

import concourse.bass as bass
import concourse.tile as tile
from concourse import mybir
from concourse.bass_utils import run_bass_kernel_spmd

B, T, C, L = 1024, 512, 128, 64
S = 2 * L + 1
NCORES = 8
BL = B // NCORES
EPS = 1e-7
NEG = -30000.0

TW = 264               # state row width: fwd s at col 2+s, bwd s at col 262-s
W = 261                # full-row window: out cols [2, 263)
NLAB = 130             # odd (label) sublattice elements: cols 3,5,...,261
NBLK = 131             # even (blank) sublattice elements: cols 2,4,...,262
LPW = 132              # per-step lp stream stride (both sublattices)
NSTEP = 256
CHUNKS = [4, 12, 48, 192]
assert sum(CHUNKS) == NSTEP
CINJ_OFF = 9999.0

F32 = mybir.dt.float32
BF16 = mybir.dt.bfloat16
ALU = mybir.AluOpType

SP_C0 = 0.8129
SP_C1 = -0.2261
INJ_BIG = -3.0e38

_MAX_WAITS = 1


def _patched_drain_and_barrier(self, tick_clock, wait_clock):
    from concourse.vector_clock import ScopedClock

    drain_inst = self.nc.sync.drain()
    wait_clock.add_sem_waits(
        drain_inst.ins, ScopedClock({None: tick_clock.global_clock})
    )
    si = drain_inst.ins.sync_info
    waits = list(si.on_wait) if si and si.on_wait else []
    if len(waits) > _MAX_WAITS:
        drain_inst.ins.sync_info = mybir.SyncInfo(
            on_wait=waits[:_MAX_WAITS], on_update=list(si.on_update or [])
        )
        for i in range(_MAX_WAITS, len(waits), _MAX_WAITS):
            extra = self.nc.sync.drain()
            extra.ins.sync_info = mybir.SyncInfo(
                on_wait=waits[i:i + _MAX_WAITS], on_update=[]
            )

    self.nc.all_engine_barrier()
    assert self.sems is not None
    popped = self.nc._tile_sem_poison_stack.pop()
    assert popped is self._sem_poison
    self.nc.clear_and_free_semaphores(list(self.sems.allocated().values()))
    self.nc.all_engine_barrier()


tile.TileContext._drain_and_barrier = _patched_drain_and_barrier


def _split_multiwait_bir(ant_bir) -> bytes:
    import json as _json

    bir = _json.loads(ant_bir)
    for f in bir.get("functions", []):
        for blk in f.get("blocks", []):
            out = []
            for ins in blk.get("instructions", []):
                si = ins.get("sync_info")
                waits = (si or {}).get("on_wait") or []
                if len(waits) > 1:
                    for j, w in enumerate(waits[:-1]):
                        out.append({
                            "debug": ins.get("debug", 0),
                            "engine": ins["engine"],
                            "ins": [],
                            "name": f"{ins['name']}_w{j}",
                            "opcode": "Drain",
                            "outs": [],
                            "sync_info": {"on_update": [], "on_wait": [w]},
                        })
                    si["on_wait"] = [waits[-1]]
                out.append(ins)
            blk["instructions"] = out
    return _json.dumps(bir).encode()


def _install_bir_splitter():
    import concourse.bass_utils as _bu
    import concourse.bass2jax as _b2j

    orig = _bu.compile_bir_kernel
    if getattr(orig, "_multiwait_patched", False):
        return

    def patched(ant_bir_str, compile_dir_path, neff_name="file.neff", **kw):
        return orig(_split_multiwait_bir(ant_bir_str), compile_dir_path,
                    neff_name=neff_name, **kw)

    patched._multiwait_patched = True
    _bu.compile_bir_kernel = patched
    if hasattr(_b2j, "compile_bir_kernel"):
        _b2j.compile_bir_kernel = patched


_install_bir_splitter()


def _lse_ref(in0, in1, s0, s1, imm2):
    a = np.asarray(in0, np.float32)
    b = np.asarray(in1, np.float32)
    m = np.maximum(a, b)
    t = m - np.minimum(a, b)
    return (m + np.maximum(s0 + s1 * t, 0.0) ** 2).astype(np.float32)


def _inj1_ref(in0, in1, s0, s1, imm2):
    a = np.asarray(in0, np.float32)
    lp = np.asarray(in1, np.float32)
    k = np.arange(a.shape[-1], dtype=np.float32)[None, :]
    u = k - (s0 if isinstance(s0, float) else np.asarray(s0, np.float32))
    inj = np.minimum(u * u, 1.0) * (s1 if isinstance(s1, float)
                                    else np.asarray(s1, np.float32))
    return (np.maximum(a, inj) + lp).astype(np.float32)


_OPS = None


def _make_ops():
    global _OPS
    if _OPS is not None:
        return _OPS
    from concourse import dve_ops as dops
    from concourse.dve_spec import (Spec, Src0, Src1, C0, C1, One, Idx,
                                    relu, sq, maxx, minn, lower)
    from concourse.dve_spec import _has_src1
    from concourse.dve_uop import DveOpSpec

    def register(name, body, ref):
        for existing in dops.OPS:
            if existing.name == name:
                return existing
        spec = Spec(body=body, reference=ref)
        row = dops._CUSTOM_DVE_ROW_BASE + len(dops.OPS)
        shas = {}
        for ver in ("v3", "v4"):
            uops = lower(spec, ver=ver)
            tmp = DveOpSpec(name=name, opcode=row, uops=uops,
                            rd1_en=_has_src1(spec))
            shas[ver] = tmp.sha(ver)
        op = dops.DveOp(name, spec, subdim=False, uops_sha=shas)
        dops.OPS.append(op)
        dops._SUB_OPCODE_FOR_NAME[name] = row
        dops.CUSTOM_DVE_SPECS[name] = spec
        return op

    m = maxx(Src0, Src1)
    n = minn(Src0, Src1)
    lse_body = m + sq(relu(C0 + C1 * (m - n)))
    lse_op = register("LSE_QSP_ANT", lse_body, _lse_ref)

    # single-column inject + emission add: out = max(Src0, w) + Src1 where
    # w = 0.0 exactly at Idx==C0 and C1=-3e38 elsewhere (C0=9999: no-op).
    u = Idx - C0
    inj1_body = maxx(Src0, minn(sq(u), One) * C1) + Src1
    inj1_op = register("INJ1_ANT", inj1_body, _inj1_ref)

    _OPS = (lse_op, inj1_op)
    return _OPS


_cached_nc = None


def build_bass():
    lse_op, inj1_op = _make_ops()
    nc = bass.Bass()
    lpl_d = nc.declare_dram_parameter("lpl", [BL, NSTEP * LPW], BF16, isOutput=False)
    lpb_d = nc.declare_dram_parameter("lpb", [BL, NSTEP * LPW], BF16, isOutput=False)
    rep_d = nc.declare_dram_parameter("rep", [BL, LPW], F32, isOutput=False)
    x0_d = nc.declare_dram_parameter("x0", [BL, TW], F32, isOutput=False)
    cll_d = nc.declare_dram_parameter("cll", [BL, NSTEP], F32, isOutput=False)
    cle_d = nc.declare_dram_parameter("cle", [BL, NSTEP], F32, isOutput=False)
    out_d = nc.declare_dram_parameter("out", [BL, 1], F32, isOutput=True)

    with tile.TileContext(nc) as tc:
        with (
            tc.tile_pool(name="lpp", bufs=1) as lp_pool,
            tc.tile_pool(name="persist", bufs=1) as pp,
        ):
            x_a = pp.tile([BL, TW], F32, tag="x_a")
            x_b = pp.tile([BL, TW], F32, tag="x_b")
            lrx_a = pp.tile([BL, LPW], F32, tag="lrx_a")
            lrx_b = pp.tile([BL, LPW], F32, tag="lrx_b")
            l1t = pp.tile([BL, TW], F32, tag="l1t")
            l2t = pp.tile([BL, LPW], F32, tag="l2t")
            rept = pp.tile([BL, LPW], F32, tag="rept")
            cllt = pp.tile([BL, NSTEP], F32, tag="cllt")
            clet = pp.tile([BL, NSTEP], F32, tag="clet")
            am = pp.tile([BL, 136], F32, tag="am")
            sc = pp.tile([BL, 176], F32, tag="sc")
            loss = pp.tile([BL, 1], F32, tag="loss")

            nc.vector.memset(x_b[:, :], NEG)
            nc.vector.memset(l1t[:, :], NEG)
            nc.vector.memset(am[:, :], NEG)
            nc.vector.memset(sc[:, :], NEG)
            nc.sync.dma_start(out=x_a[:, :], in_=x0_d[:, :])
            # small tables issue from the idle ACT queue so the lp chunk
            # issue on SP isn't serialized behind them
            nc.scalar.dma_start(out=rept[:, :], in_=rep_d[:, :])
            nc.scalar.dma_start(out=cllt[:, :], in_=cll_d[:, :])
            nc.scalar.dma_start(out=clet[:, :], in_=cle_d[:, :])
            lpts = []
            lo = 0
            for ci, csz in enumerate(CHUNKS):
                # lpb is consumed earlier in each step than lpl
                lpbt = lp_pool.tile([BL, csz * LPW], BF16, tag=f"lpb{ci}")
                nc.sync.dma_start(out=lpbt[:, :],
                                  in_=lpb_d[:, lo * LPW:(lo + csz) * LPW])
                lplt = lp_pool.tile([BL, csz * LPW], BF16, tag=f"lpl{ci}")
                nc.sync.dma_start(out=lplt[:, :],
                                  in_=lpl_d[:, lo * LPW:(lo + csz) * LPW])
                lpts.append((lplt, lpbt, lo, csz))
                lo += csz

            xc, xn = x_a, x_b
            lrc, lrn = lrx_a, lrx_b
            for lplt, lpbt, lo, csz in lpts:
                for il in range(csz):
                    i = lo + il
                    # early-step wavefront narrowing: fwd mass lives at
                    # s <= 2i+1 (cols <= 3+2i) and bwd mass at s >= 63-2i
                    # (cols <= 199+2i), so all windows can stop at We and
                    # grow by 2 cols/step until full (unwritten xn cols keep
                    # stale NEG-ish values, which read as -inf).
                    We = min(2 + W, 206 + 2 * i)
                    ne = len(range(134, We, 2))
                    no = len(range(3, We, 2))
                    # gated label skip path on GPSIMD (odd sublattice); only
                    # Add/Mult lower to Pool-legal ISA opcodes, so gate by
                    # adding {0, NEG}. Runs in the shadow of the DVE ops.
                    nc.gpsimd.tensor_add(lrc[:, 0:no], xc[:, 1:We - 2:2],
                                         rept[:, 0:no])
                    # DVE: one LSE2 serves both parities. While the fwd
                    # wavefront (s <= 2i+1, cols <= 3+2i) is far from the
                    # bwd region, split around the dead middle: the saved
                    # elements beat the extra instruction init for i<=33.
                    Wf = 7 + 2 * i
                    if 132 - Wf > 58:
                        nc.vector._custom_dve(lse_op, out=l1t[:, 2:Wf],
                                              in0=xc[:, 2:Wf],
                                              in1=xc[:, 1:Wf - 1],
                                              s0=SP_C0, s1=SP_C1)
                        nc.vector._custom_dve(lse_op, out=l1t[:, 132:We],
                                              in0=xc[:, 132:We],
                                              in1=xc[:, 131:We - 1],
                                              s0=SP_C0, s1=SP_C1)
                    else:
                        nc.vector._custom_dve(lse_op, out=l1t[:, 2:We],
                                              in0=xc[:, 2:We],
                                              in1=xc[:, 1:We - 1],
                                              s0=SP_C0, s1=SP_C1)
                    # fwd-even emission add on GPSIMD (no injections there)
                    nc.gpsimd.tensor_add(xn[:, 2:131:2], l1t[:, 2:131:2],
                                         lpbt[:, il * LPW: il * LPW + 65])
                    # bwd-even: inject + emission on DVE
                    nc.vector._custom_dve(inj1_op, out=xn[:, 134:We:2],
                                          in0=l1t[:, 134:We:2],
                                          in1=lpbt[:, il * LPW + 66:
                                                   il * LPW + 66 + ne],
                                          s0=clet[:, i:i + 1], s1=INJ_BIG)
                    # labels: second LSE2 against the gated skip path
                    nc.vector._custom_dve(lse_op, out=l2t[:, 0:no],
                                          in0=l1t[:, 3:We:2],
                                          in1=lrc[:, 0:no],
                                          s0=SP_C0, s1=SP_C1)
                    nc.vector._custom_dve(inj1_op, out=xn[:, 3:We:2],
                                          in0=l2t[:, 0:no],
                                          in1=lplt[:, il * LPW: il * LPW + no],
                                          s0=cllt[:, i:i + 1], s1=INJ_BIG)
                    xc, xn = xn, xc
                    lrc, lrn = lrn, lrc

            # readout: alpha[s] at col 2+s, beta[s] at col 262-s
            nc.vector.tensor_add(am[:, 0:S], xc[:, 2:2 + S],
                                 xc[:, 262:133:-1])

            def tree(out_o, in_t, in_o, wlo):
                nc.vector._custom_dve(
                    lse_op, out=sc[:, out_o:out_o + wlo],
                    in0=in_t[:, in_o:in_o + wlo],
                    in1=in_t[:, in_o + wlo:in_o + 2 * wlo],
                    s0=SP_C0, s1=SP_C1)

            tree(0, am, 0, 65)      # 129 -> 65  (am[129]=NEG)
            tree(80, sc, 0, 33)     # 65 -> 33   (sc[65]=NEG)
            tree(120, sc, 80, 17)   # 33 -> 17   (sc[113]=NEG)
            tree(140, sc, 120, 9)   # 17 -> 9    (sc[137]=NEG)
            tree(152, sc, 140, 5)   # 9 -> 5     (sc[149]=NEG)
            tree(160, sc, 152, 3)   # 5 -> 3     (sc[157]=NEG)
            tree(168, sc, 160, 2)   # 3 -> 2     (sc[163]=NEG)
            tree(172, sc, 168, 1)   # 2 -> 1
            nc.vector.tensor_scalar_mul(loss[:, 0:1], sc[:, 172:173], -1.0)
            nc.sync.dma_start(out=out_d[:, :], in_=loss[:, 0:1])
    mybir.codegen_inst_isa_subclasses(nc)
    return nc


def _host_prep(y_pred, labels, input_length, label_length):
    blank = C - 1
    lab = labels.astype(np.int64)
    q_l = np.take_along_axis(y_pred, lab[:, None, :], axis=2)   # [B,T,64]
    lp_l = np.log(q_l.astype(np.float32) + EPS)                 # label lp
    lp_b = np.log(y_pred[:, :, blank].astype(np.float32) + EPS) # [B,T] blank lp
    frozen = np.arange(T)[None, :] >= input_length[:, None]
    lp_l[frozen, :] = 0.0
    lp_b[frozen] = 0.0

    # rep gate for label k vs k-1 (fwd: gates l_{k-1} -> l_k)
    rep = np.full((B, L), 0.0, np.float32)
    rep[:, 1:] = np.where(labels[:, 1:] != labels[:, :-1], 0.0, NEG)
    # rep[:, 0] = 0.0: the gated read hits the NEG pad anyway.

    lens = input_length.astype(np.int64)
    llen = label_length.astype(np.int64)                        # [B] in [32,64]

    # odd-sublattice lp stream: e=0..63 fwd labels (out col 3+2e, s=1+2e),
    # e=64,65 pads, e=66..129 bwd labels (s=259-2e -> k=129-e), lp[510-i]
    lpl = np.full((B, NSTEP, LPW), NEG, np.float32)
    lpl[:, :, 0:64] = lp_l[:, 0:NSTEP, :]
    lpl[:, 0:NSTEP - 1, 66:130] = lp_l[:, 510:255:-1, ::-1]
    lpl[:, NSTEP - 1, 66:130] = 0.0
    lpl = lpl.reshape(B, NSTEP * LPW).astype(ml_dtypes.bfloat16)

    # even-sublattice lp stream: e=0..64 fwd blanks (col 2+2e), e=65 pad
    # (col 132), e=66..130 bwd blanks (col 2+2e = 134..262)
    lpb = np.full((B, NSTEP, LPW), NEG, np.float32)
    lpb[:, :, 0:65] = lp_b[:, 0:NSTEP, None]
    lpb[:, 0:NSTEP - 1, 66:131] = lp_b[:, 510:255:-1, None]
    lpb[:, NSTEP - 1, 66:131] = 0.0
    lpb = lpb.reshape(B, NSTEP * LPW).astype(ml_dtypes.bfloat16)

    # rep stream on the odd sublattice: e=0..63 fwd rep_e; e=64..66 pads;
    # e=67..129 bwd label k=129-e gated by rep_{k+1} = rep_{130-e}
    repc = np.full((B, LPW), NEG, np.float32)
    repc[:, 0:64] = rep
    repc[:, 67:130] = rep[:, 1:64][:, ::-1]

    # initial state: fwd s at col 2+s, bwd s at col 262-s
    x0 = np.full((B, TW), NEG, np.float32)
    x0[:, 2] = 0.0                                              # alpha seed s=0
    bi = np.nonzero(lens == 512)[0]
    x0[bi, 262 - 2 * llen[bi]] = lp_b[bi, 511]                  # s_last (blank)
    x0[bi, 263 - 2 * llen[bi]] = lp_l[bi, 511, llen[bi] - 1]    # s_last-1
    # injection tables: odd sublattice element 130-llen; even sublattice's
    # narrowed bwd window starts at element 65, so its Idx = 65-llen
    cl = np.full((B, NSTEP), CINJ_OFF, np.float32)
    cle = np.full((B, NSTEP), CINJ_OFF, np.float32)
    ii = 511 - lens
    has = (ii >= 0) & (ii <= 255)
    bi = np.nonzero(has)[0]
    cl[bi, ii[bi]] = (130 - llen[bi]).astype(np.float32)
    cle[bi, ii[bi]] = (64 - llen[bi]).astype(np.float32)

    return lpl, lpb, repc, x0, cl, cle


def kernel(y_pred, labels, input_length, label_length):
    global _cached_nc
    y_pred = np.asarray(y_pred, np.float32)
    labels = np.asarray(labels, np.int32)
    input_length = np.asarray(input_length, np.int32)
    label_length = np.asarray(label_length, np.int32)
    lpl, lpb, repc, x0, cl, cle = _host_prep(
        y_pred, labels, input_length, label_length)
    if _cached_nc is None:
        _cached_nc = build_bass()
    in_maps = []
    for i in range(NCORES):
        sl = slice(i * BL, (i + 1) * BL)
        in_maps.append({"lpl": lpl[sl], "lpb": lpb[sl], "rep": repc[sl],
                        "x0": x0[sl], "cll": cl[sl], "cle": cle[sl]})
    res = run_bass_kernel_spmd(_cached_nc, in_maps, list(range(NCORES)))
    out = np.concatenate([res.results[i]["out"] for i in range(NCORES)], axis=0)
    return out.astype(np.float32)
